# revision 1
# baseline (speedup 1.0000x reference)
"""Trainium2 Bass kernel: 16-head MHA (B=2, T=2048, D=1024, d_k=64).

Sharding (8 NeuronCores): data-parallel over the batch (2) x tensor-parallel
over head groups (4 groups of 4 heads).  Core c handles batch b = c//4 and
heads [4g, 4g+4) with g = c%4.  Each core computes its partial output
    sum_{h in group} softmax((q Wq_h + bq_h)(k Wk_h)^T / 8) (v Wv_h) Wo_h
and the host sums the 4 partials per batch and adds the constant row
bo + bv @ Wo once.  bk is dropped: with the all-ones mask it shifts every
score row by a per-row constant, which softmax ignores exactly.

Numerics: activations/weights stream as bf16 into the PE (fp32 PSUM
accumulation everywhere); softmax denominators stay float32r and 1/x is
computed as exp(-ln(x)) on the scalar engine in one batched shot per
q-group (single ACT table set).  End-to-end absmax error is a few 1e-3
relative - the same class as running the reference itself in bf16.

Per-core pipeline:
  1. Q^T, K^T, V^T projections from host-pretransposed X^T bf16 chunks
     streamed over HBM, contracted on the PE (bf16 in, fp32 PSUM out).
     Q^T/K^T live as two [128, 2048] head-pair tiles (head h on
     partitions (h%2)*64..); V^T is transposed back on the PE into 16
     [128, 260] "V_ext" tiles: per head 64 V columns plus a ones column
     that yields the softmax row sums for free in the attention*V matmul.
  2. Per (head pair, 1024-wide q slice): scores transposed
     S'[k, q] = K Q^T - the two heads' matmuls are emitted adjacently on
     disjoint PE row groups; exp on the scalar engine (1/8 scale folded
     in); O^T accumulated over the 16 k tiles with the row-sum row.
  3. Row sums collect on one partition; one batched Ln+Exp computes the
     reciprocals; a rank-1 PE matmul broadcasts them across partitions;
     DVE normalizes into two head-pair-stacked [128, 2048] O^T tiles
     (odd heads land on partitions 64:128 via the DVE write base).
  4. Output projection: per 128-row tile, two C=128 bf16 matmuls against
     head-pair-stacked Wo tiles, fp32 drain, DMA out.
"""

import functools
import os

import ml_dtypes
import numpy as np

import concourse.bass as bass
import concourse.mybir as mybir
import concourse.tile as tile
from concourse import bacc
from concourse.bass_utils import run_bass_kernel_spmd
from concourse.masks import make_identity

F32 = mybir.dt.float32
F32R = mybir.dt.float32r
BF16 = mybir.dt.bfloat16
AFT = mybir.ActivationFunctionType
BF = ml_dtypes.bfloat16

D = 1024          # model dim
T = 2048          # sequence length
B = 2             # batch
HEADS = 16        # total heads
DK = 64           # head dim
NCORES = 8
GH = 4            # heads per core
GD = GH * DK      # 256 projection cols per core
NF = D // 128     # 8 contraction chunks
NKT = T // 128    # 16 k/t tiles
SCALE = 1.0 / np.sqrt(np.float32(DK))  # 1/8

# Results of the last run (for test harness introspection: exec_time_ns etc.)
LAST_RESULTS = None


@functools.lru_cache(maxsize=1)
def _build_program():
    nc = bacc.Bacc("TRN2", target_bir_lowering=False, debug=False,
                   num_devices=NCORES)

    xqT = nc.declare_dram_parameter("xqT", [D, T], BF16, isOutput=False)
    xkT = nc.declare_dram_parameter("xkT", [D, T], BF16, isOutput=False)
    xvT = nc.declare_dram_parameter("xvT", [D, T], BF16, isOutput=False)
    wq = nc.declare_dram_parameter("wq", [128, NF * GD], BF16, isOutput=False)
    wk = nc.declare_dram_parameter("wk", [128, NF * GD], BF16, isOutput=False)
    wv = nc.declare_dram_parameter("wv", [128, NF * GD], BF16, isOutput=False)
    wo = nc.declare_dram_parameter("wo", [2, 128, D], F32R, isOutput=False)
    bqv = nc.declare_dram_parameter("bqv", [128, 2], F32, isOutput=False)
    out = nc.declare_dram_parameter("out", [T, D], F32, isOutput=True)

    with tile.TileContext(nc) as tc:
        # ---- persistent pools -------------------------------------------
        with (
            tc.tile_pool(name="qk", bufs=4) as qk_pool,
            tc.tile_pool(name="vext", bufs=NKT) as vext_pool,
            tc.tile_pool(name="wop", bufs=2) as wo_pool,
            tc.tile_pool(name="otp", bufs=2) as ot_pool,
            tc.tile_pool(name="const", bufs=1) as const_pool,
        ):
            ident = const_pool.tile([128, 128], BF16, tag="ident")
            make_identity(nc, ident[:])
            ones_f32 = const_pool.tile([128, DK], F32, tag="ones32")
            nc.gpsimd.memset(ones_f32[:], 1.0)
            ones_sb = const_pool.tile([1, DK], F32R, tag="ones")
            nc.vector.tensor_copy(ones_sb[:], ones_f32[0:1, :])
            bqv_sb = const_pool.tile([128, 2], F32, tag="bqv")
            nc.sync.dma_start(bqv_sb[:], bqv[:])

            QT = [qk_pool.tile([128, T], F32R, tag="qk", name=f"qt{m}")
                  for m in range(2)]
            KT = [qk_pool.tile([128, T], F32R, tag="qk", name=f"kt{m}")
                  for m in range(2)]
            VE = [vext_pool.tile([128, GH * (DK + 1)], F32R, tag="vext",
                                 name=f"ve{i}") for i in range(NKT)]
            WO = [wo_pool.tile([128, D], F32R, tag="wop", name=f"wo{m}")
                  for m in range(2)]
            OT = [ot_pool.tile([128, T], F32R, tag="ot", name=f"ot{m}")
                  for m in range(2)]

            # ---- phase A: projections -----------------------------------
            with (
                tc.tile_pool(name="wts", bufs=3) as w_pool,
                tc.tile_pool(name="xt", bufs=4) as xt_pool,
                tc.tile_pool(name="vt", bufs=2) as vt_pool,
                tc.tile_pool(name="psA", bufs=8,
                             space=bass.MemorySpace.PSUM) as psA,
            ):
                VT = [vt_pool.tile([128, T], BF16, tag="vt", name=f"vt{m}")
                      for m in range(2)]

                def projection(w_dram, x_dram, drain):
                    w_sb = w_pool.tile([128, NF * GD], BF16, tag="w")
                    nc.sync.dma_start(w_sb[:], w_dram[:])
                    ps = [psA.tile([128, 512], F32, tag="proj",
                                   name=f"pj{i}") for i in range(8)]
                    for fc in range(NF):
                        xt = xt_pool.tile([128, T], BF16, tag="xt")
                        nc.sync.dma_start(
                            xt[:], x_dram[fc * 128:(fc + 1) * 128, :])
                        for m in range(2):
                            for qh in range(4):
                                nc.tensor.matmul(
                                    ps[m * 4 + qh][:],
                                    w_sb[:, fc * GD + m * 128:
                                         fc * GD + (m + 1) * 128],
                                    xt[:, qh * 512:(qh + 1) * 512],
                                    start=(fc == 0), stop=(fc == NF - 1))
                    for m in range(2):
                        for qh in range(4):
                            drain(m, qh, ps[m * 4 + qh])

                def q_drain(m, qh, ps):
                    nc.vector.tensor_scalar_add(
                        QT[m][:, qh * 512:(qh + 1) * 512], ps[:],
                        bqv_sb[:, m:m + 1])

                def k_drain(m, qh, ps):
                    nc.vector.tensor_copy(
                        KT[m][:, qh * 512:(qh + 1) * 512], ps[:])

                def v_drain(m, qh, ps):
                    nc.vector.tensor_copy(
                        VT[m][:, qh * 512:(qh + 1) * 512], ps[:])

                projection(wq, xqT, q_drain)
                projection(wk, xkT, k_drain)
                projection(wv, xvT, v_drain)

                # V^T -> V_ext (PE transpose of 128x128 blocks, per pair)
                for kt in range(NKT):
                    ve = VE[kt]
                    ve_r = ve[:].rearrange("p (h x) -> p h x", x=DK + 1)
                    nc.vector.tensor_copy(
                        ve_r[:, :, DK:DK + 1],
                        ones_f32[:, 0:GH].rearrange("p (h x) -> p h x", x=1))
                    for m in range(2):
                        tp = psA.tile([128, 128], BF16, tag="proj")
                        nc.tensor.transpose(
                            tp[:], VT[m][:, kt * 128:(kt + 1) * 128],
                            ident[:])
                        nc.vector.tensor_copy(
                            ve_r[:, 2 * m:2 * m + 2, 0:DK],
                            tp[:].rearrange("k (h j) -> k h j", j=DK))

            nc.sync.dma_start(WO[0][:], wo[0])
            nc.sync.dma_start(WO[1][:], wo[1])

            # ---- phase B: attention -------------------------------------
            with (
                tc.tile_pool(name="ep", bufs=10) as epool,
                tc.tile_pool(name="ubp", bufs=8) as ub_pool,
                tc.tile_pool(name="rsp", bufs=1) as rs_pool,
                tc.tile_pool(name="rsbp", bufs=2) as rsb_pool,
                tc.tile_pool(name="psS", bufs=2,
                             space=bass.MemorySpace.PSUM) as psS,
                tc.tile_pool(name="psO", bufs=2,
                             space=bass.MemorySpace.PSUM) as psO,
            ):
                for qs in range(2):          # 1024-wide q slices
                    q0 = qs * 1024
                    # rowsums of all 8 (head, half) pairs of this group,
                    # collected on one partition for a single batched 1/x
                    rs = rs_pool.tile([1, 8 * 512], F32R, tag="rs")
                    ub = {}
                    for hp in range(2):      # head pairs -> PE row groups
                        o_ps = [psO.tile([128, 1024], F32, tag="o",
                                         name=f"o{i}") for i in range(2)]
                        for kt in range(NKT):
                            ss = [psS.tile([128, 1024], F32, tag="s",
                                           name=f"s{i}") for i in range(2)]
                            # the two heads' scores matmuls are adjacent:
                            # same rhs columns, disjoint row groups
                            for hf in range(2):
                                for hh in range(2):
                                    lo = hh * DK
                                    nc.tensor.matmul(
                                        ss[hh][:, hf * 512:(hf + 1) * 512],
                                        KT[hp][lo:lo + DK,
                                               kt * 128:(kt + 1) * 128],
                                        QT[hp][lo:lo + DK,
                                               q0 + hf * 512:
                                               q0 + (hf + 1) * 512],
                                        start=True, stop=True)
                            es = []
                            for hh in range(2):
                                e = epool.tile([128, 1024], F32R, tag="e")
                                nc.scalar.activation(e[:], ss[hh][:],
                                                     AFT.Exp,
                                                     scale=float(SCALE))
                                es.append(e)
                            for hh in range(2):
                                h = hp * 2 + hh
                                for hf in range(2):
                                    nc.tensor.matmul(
                                        o_ps[hh][0:DK + 1,
                                                 hf * 512:(hf + 1) * 512],
                                        VE[kt][:, h * (DK + 1):
                                               (h + 1) * (DK + 1)],
                                        es[hh][:, hf * 512:(hf + 1) * 512],
                                        start=(kt == 0),
                                        stop=(kt == NKT - 1))
                        for hh in range(2):
                            h = hp * 2 + hh
                            for hf in range(2):
                                i8 = h * 2 + hf
                                u = ub_pool.tile([DK, 512], F32, tag="ub",
                                                 name=f"ub{i8}")
                                nc.vector.tensor_copy(
                                    u[:], o_ps[hh][0:DK,
                                                   hf * 512:(hf + 1) * 512])
                                nc.vector.tensor_copy(
                                    rs[0:1, i8 * 512:(i8 + 1) * 512],
                                    o_ps[hh][DK:DK + 1,
                                             hf * 512:(hf + 1) * 512])
                                ub[i8] = u

                    # one batched reciprocal for the whole group: 1/x via
                    # exp(-ln(x)), in place on the collected rowsums
                    nc.scalar.activation(rs[:], rs[:], AFT.Ln)
                    nc.scalar.activation(rs[:], rs[:], AFT.Exp, scale=-1.0)

                    for i8 in range(8):
                        h, hf = divmod(i8, 2)
                        m, lo = h // 2, (h % 2) * DK
                        r_ps = psS.tile([128, 1024], F32, tag="s",
                                        name=f"rps{i8}")
                        nc.tensor.matmul(
                            r_ps[0:DK, 0:512], ones_sb[:],
                            rs[0:1, i8 * 512:(i8 + 1) * 512],
                            start=True, stop=True)
                        r_sb = rsb_pool.tile([DK, 512], F32, tag="rsb")
                        nc.vector.tensor_copy(r_sb[:], r_ps[0:DK, 0:512])
                        # odd heads land on partitions 64:128 of the
                        # head-pair-stacked O^T tile via the DVE write base
                        nc.vector.tensor_mul(
                            OT[m][lo:lo + DK,
                                  q0 + hf * 512:q0 + (hf + 1) * 512],
                            ub[i8][:], r_sb[:])

            # ---- output projection tail ---------------------------------
            with (
                tc.tile_pool(name="osbp", bufs=4) as out_pool,
                tc.tile_pool(name="psF", bufs=3,
                             space=bass.MemorySpace.PSUM) as psF,
            ):
                for tt in range(NKT):
                    osb = out_pool.tile([128, 1024], F32, tag="osb")
                    f_ps = psF.tile([128, 1024], F32, tag="f")
                    for ei in range(2):
                        for m in range(2):
                            nc.tensor.matmul(
                                f_ps[:, ei * 512:(ei + 1) * 512],
                                OT[m][:, tt * 128:(tt + 1) * 128],
                                WO[m][:, ei * 512:(ei + 1) * 512],
                                start=(m == 0), stop=(m == 1))
                    nc.vector.tensor_copy(osb[:], f_ps[:])
                    nc.sync.dma_start(
                        out[tt * 128:(tt + 1) * 128, :], osb[:])

    from concourse.bacc import get_activation_tables
    import bass_rust as _br
    _combined = "natural_log_exp_and_others"
    _tabs = []
    for _name, _fns in get_activation_tables(nc.m.arch).items():
        if _name != _combined:
            _fns = _fns - {AFT.Exp, AFT.Ln}
        _tabs.append((_name, _fns))
    _br.insert_act_table_loads(nc, _tabs)
    nc.compile()
    return nc


def _numpy_reference(q, k, v, mask, Wq, bq, Wk, bk, Wv, bv, Wo, bo):
    """Fallback for a non-trivial mask (never hit with the stock inputs)."""
    Bn, Tn, _ = q.shape
    H, dk = HEADS, DK

    def split(x):
        return x.reshape(Bn, Tn, H, dk).transpose(0, 2, 1, 3)

    qh = split(q @ Wq + bq)
    kh = split(k @ Wk + bk)
    vh = split(v @ Wv + bv)
    s = np.einsum("bhqd,bhkd->bhqk", qh, kh) / np.sqrt(np.float32(dk))
    s = np.where(mask, s, -np.inf)
    s = s - s.max(axis=-1, keepdims=True)
    e = np.exp(s)
    a = e / e.sum(axis=-1, keepdims=True)
    o = np.einsum("bhqk,bhkd->bhqd", a, vh)
    o = o.transpose(0, 2, 1, 3).reshape(Bn, Tn, H * dk)
    return (o @ Wo + bo).astype(np.float32)


def kernel(q, k, v, mask, Wq, bq, Wk, bk, Wv, bv, Wo, bo):
    global LAST_RESULTS
    q = np.asarray(q, np.float32)
    k = np.asarray(k, np.float32)
    v = np.asarray(v, np.float32)
    mask = np.asarray(mask, bool)
    Wq, bq = np.asarray(Wq, np.float32), np.asarray(bq, np.float32)
    Wk, bk = np.asarray(Wk, np.float32), np.asarray(bk, np.float32)
    Wv, bv = np.asarray(Wv, np.float32), np.asarray(bv, np.float32)
    Wo, bo = np.asarray(Wo, np.float32), np.asarray(bo, np.float32)

    if not mask.all():
        return _numpy_reference(q, k, v, mask, Wq, bq, Wk, bk, Wv, bv, Wo, bo)

    nc = _build_program()

    # host-side sharding
    xT = {}
    for b in range(B):
        xT[b] = tuple(np.ascontiguousarray(x[b].T.astype(BF))
                      for x in (q, k, v))

    def w_chunks(W, g):
        # (1024, 256) head-group slice -> [128, 8*256] chunk-major layout
        Wg = W[:, g * GD:(g + 1) * GD]
        return np.ascontiguousarray(
            Wg.reshape(NF, 128, GD).transpose(1, 0, 2)
            .reshape(128, NF * GD).astype(BF))

    in_maps = []
    for c in range(NCORES):
        b, g = divmod(c, GH)
        xq_t, xk_t, xv_t = xT[b]
        in_maps.append({
            "xqT": xq_t, "xkT": xk_t, "xvT": xv_t,
            "wq": w_chunks(Wq, g), "wk": w_chunks(Wk, g),
            "wv": w_chunks(Wv, g),
            "wo": np.ascontiguousarray(
                Wo[g * GD:(g + 1) * GD, :]).reshape(2, 128, D),
            "bqv": np.ascontiguousarray(
                bq[g * GD:(g + 1) * GD].reshape(2, 128).T),
        })

    LAST_RESULTS = run_bass_kernel_spmd(
        nc, in_maps, list(range(NCORES)),
        trace=bool(os.environ.get("KERNEL_TRACE")))
    res = LAST_RESULTS.results

    const_row = (bv @ Wo + bo).astype(np.float32)  # attn rows sum to 1
    full = np.empty((B, T, D), np.float32)
    for b in range(B):
        acc = res[b * GH]["out"].astype(np.float32)
        for g in range(1, GH):
            acc = acc + res[b * GH + g]["out"]
        full[b] = acc + const_row
    return full



# revision 13
# speedup vs baseline: 1.6792x; 1.6792x over previous
"""Trainium2 Bass kernel: 16-head MHA (B=2, T=2048, D=1024, d_k=64).

Sharding (8 NeuronCores): data-parallel over the batch (2) x tensor-parallel
over head groups (4 groups of 4 heads).  Core c handles batch b = c//4 and
heads [4g, 4g+4) with g = c%4.  Each core computes its partial output
    sum_{h in group} softmax((q Wq_h + bq_h)(k Wk_h)^T / 8) (v Wv_h) Wo_h
and the host sums the 4 partials per batch and adds the constant row
bo + bv @ Wo once.  bk is dropped: with the all-ones mask it shifts every
score row by a per-row constant, which softmax ignores exactly.

v2 design notes (vs the 391us baseline):
  * every matmul operand is bf16 (FWL weight loads, fp32 PSUM accumulate);
    output DMA'd as bf16 and upconverted host-side.
  * V is projected directly in [t, v-col] layout (stationary = x^T chunk,
    moving = Wv), killing the 32 PE transposes + drains of v1.
  * attention runs in 512-wide q stripes; per (stripe, head-pair, kt):
    2 scores MMs -> one [128,1024] fp32 PSUM tile, one ACT exp -> bf16,
    2 aV MMs accumulating into per-head [65,512] PSUM.  PSUM budget:
    scores dbuf 2x2 banks + O-accum 2 + filler 2 = 8 banks exactly, so
    scores(kt+1) / exp(kt) / aV(kt) pipeline without stalls.
  * Q projection of stripe s+1, the reciprocal dance, and the output
    projection of stripe s-1 are emitted as PE fillers inside the
    (ACT-bound) attention loop, keeping the PE dense so the HAM clock
    gate stays at 2.4 GHz instead of the baseline's 1.2 GHz cold clock.
  * softmax denominators: rowsums ride along as a 65th V_ext column; the
    reciprocal is computed partition-parallel by DVE 32-block transposing
    the per-head rowsum rows (heads pinned to partitions 0/32/64/96),
    one batched ACT Ln + Exp(-1), transposing back, and broadcasting
    across partitions with per-head rank-1 bf16 matmuls.
"""

import functools
import os
from collections import deque

import ml_dtypes
import numpy as np

import concourse.bass as bass
import concourse.mybir as mybir
import concourse.tile as tile
from concourse import bacc
from concourse.bass_utils import run_bass_kernel_spmd

F32 = mybir.dt.float32
F32R = mybir.dt.float32r
BF16 = mybir.dt.bfloat16
AFT = mybir.ActivationFunctionType
BF = ml_dtypes.bfloat16

D = 1024          # model dim
T = 2048          # sequence length
B = 2             # batch
HEADS = 16        # total heads
DK = 64           # head dim
NCORES = 8
GH = 4            # heads per core
GD = GH * DK      # 256 projection cols per core
NF = D // 128     # 8 contraction chunks
NKT = T // 128    # 16 k tiles
NQS = 4           # 512-wide q stripes
QW = T // NQS     # 512
SCALE = 1.0 / np.sqrt(np.float32(DK))  # 1/8

# Results of the last run (for test harness introspection: exec_time_ns etc.)
LAST_RESULTS = None


@functools.lru_cache(maxsize=1)
def _build_program():
    nc = bacc.Bacc("TRN2", target_bir_lowering=False, debug=False,
                   num_devices=NCORES)

    xqT = nc.declare_dram_parameter("xqT", [D, T], BF16, isOutput=False)
    xkT = nc.declare_dram_parameter("xkT", [D, T], BF16, isOutput=False)
    xvT = nc.declare_dram_parameter("xvT", [D, T], BF16, isOutput=False)
    wq = nc.declare_dram_parameter("wq", [128, NF * GD], BF16, isOutput=False)
    wk = nc.declare_dram_parameter("wk", [128, NF * GD], BF16, isOutput=False)
    wv = nc.declare_dram_parameter("wv", [128, NF * GD], BF16, isOutput=False)
    wo = nc.declare_dram_parameter("wo", [2, 128, D], BF16, isOutput=False)
    bqv = nc.declare_dram_parameter("bqv", [128, 2], F32, isOutput=False)
    out = nc.declare_dram_parameter("out", [T, D], BF16, isOutput=True)

    with tile.TileContext(nc) as tc:
        # ---- persistent pools -------------------------------------------
        with (
            tc.tile_pool(name="kt", bufs=2) as kt_pool,
            tc.tile_pool(name="vext", bufs=NKT) as vext_pool,
            tc.tile_pool(name="qts", bufs=NQS * 2) as qts_pool,
            tc.tile_pool(name="ots", bufs=NQS * 2) as ots_pool,
            tc.tile_pool(name="wts", bufs=3) as w_pool,
            tc.tile_pool(name="wop", bufs=2) as wo_pool,
            tc.tile_pool(name="xq", bufs=NF) as xq_pool,
            tc.tile_pool(name="const", bufs=1) as const_pool,
        ):
            ones_f32 = const_pool.tile([128, GH], F32, tag="ones32")
            nc.gpsimd.memset(ones_f32[:], 1.0)
            ones_bf = const_pool.tile([128, DK], BF16, tag="onesbf")
            nc.gpsimd.memset(ones_bf[:], 1.0)
            # head-pair selectors: sel[hp][c, m] = (c == 32*(hp*2 + m//64)),
            # i.e. r_bc[m, :] = rinvT[32*head(m), :] after the C=128 matmul
            sel = [const_pool.tile([128, 128], BF16, tag=f"sel{hp}",
                                   name=f"sel{hp}") for hp in range(2)]
            for hp in range(2):
                nc.gpsimd.memset(sel[hp][:], 0.0)
                for hh in range(2):
                    c = 32 * (hp * 2 + hh)
                    nc.vector.tensor_copy(
                        sel[hp][c:c + 1, hh * DK:(hh + 1) * DK],
                        ones_bf[0:1, 0:DK])
            bqv_sb = const_pool.tile([128, 2], F32, tag="bqv")
            nc.sync.dma_start(bqv_sb[:], bqv[:])

            KT = [kt_pool.tile([128, T], BF16, tag="kt", name=f"kt{m}")
                  for m in range(2)]
            VE = [vext_pool.tile([128, GH * (DK + 1)], BF16, tag="vext",
                                 name=f"ve{i}") for i in range(NKT)]
            # per-stripe Q^T and O^T tiles (heads of pair hp stacked 64+64)
            QTs = [[qts_pool.tile([128, QW], BF16, tag="qts",
                                  name=f"qt{s}_{m}") for m in range(2)]
                   for s in range(NQS)]
            OTs = [[ots_pool.tile([128, QW], BF16, tag="ots",
                                  name=f"ot{s}_{m}") for m in range(2)]
                   for s in range(NQS)]
            WO = [wo_pool.tile([128, D], BF16, tag="wop", name=f"wo{m}")
                  for m in range(2)]

            wq_sb = w_pool.tile([128, NF * GD], BF16, tag="w", name="wq_sb")
            wk_sb = w_pool.tile([128, NF * GD], BF16, tag="w", name="wk_sb")
            wv_sb = w_pool.tile([128, NF * GD], BF16, tag="w", name="wv_sb")
            XQ = [xq_pool.tile([128, T], BF16, tag="xq", name=f"xq{i}")
                  for i in range(NF)]

            # ---- DMA in (ordered: K needs first, then V, then Q) --------
            nc.sync.dma_start(wk_sb[:], wk[:])
            xk_tiles = []
            xv_tiles = []
            with (
                tc.tile_pool(name="xkv", bufs=2 * NF) as xkv_pool,
                tc.tile_pool(name="psA", bufs=8,
                             space=bass.MemorySpace.PSUM) as psA,
            ):
                for fc in range(NF):
                    xk_t = xkv_pool.tile([128, T], BF16, tag="xkv",
                                         name=f"xk{fc}")
                    nc.sync.dma_start(xk_t[:], xkT[fc * 128:(fc + 1) * 128, :])
                    xk_tiles.append(xk_t)
                nc.sync.dma_start(wv_sb[:], wv[:])
                for fc in range(NF):
                    xv_t = xkv_pool.tile([128, T], BF16, tag="xkv",
                                         name=f"xv{fc}")
                    nc.sync.dma_start(xv_t[:], xvT[fc * 128:(fc + 1) * 128, :])
                    xv_tiles.append(xv_t)
                nc.sync.dma_start(wq_sb[:], wq[:])
                for fc in range(NF):
                    nc.sync.dma_start(XQ[fc][:], xqT[fc * 128:(fc + 1) * 128, :])
                nc.sync.dma_start(WO[0][:], wo[0])
                nc.sync.dma_start(WO[1][:], wo[1])

                # ---- K projection (transposed layout) -------------------
                ps_k = [psA.tile([128, QW], F32, tag="psA", name=f"psk{i}")
                        for i in range(8)]
                for fc in range(NF):
                    for m in range(2):
                        for qh in range(4):
                            nc.tensor.matmul(
                                ps_k[m * 4 + qh][:],
                                wk_sb[:, fc * GD + m * 128:
                                      fc * GD + (m + 1) * 128],
                                xk_tiles[fc][:, qh * QW:(qh + 1) * QW],
                                start=(fc == 0), stop=(fc == NF - 1))
                for m in range(2):
                    for qh in range(4):
                        nc.vector.tensor_copy(
                            KT[m][:, qh * QW:(qh + 1) * QW],
                            ps_k[m * 4 + qh][:])

                # ---- V projection, direct [t, vcol] layout --------------
                for tb in range(NKT):
                    ps_v = psA.tile([128, QW], F32, tag="psA", name=f"psv{tb}")
                    for dc in range(NF):
                        nc.tensor.matmul(
                            ps_v[:, 0:GD],
                            xv_tiles[dc][:, tb * 128:(tb + 1) * 128],
                            wv_sb[:, dc * GD:(dc + 1) * GD],
                            start=(dc == 0), stop=(dc == NF - 1))
                    ve_r = VE[tb][:].rearrange("p (h x) -> p h x", x=DK + 1)
                    nc.vector.tensor_copy(
                        ve_r[:, :, 0:DK],
                        ps_v[:, 0:GD].rearrange("p (h x) -> p h x", x=DK))
                    nc.vector.tensor_copy(
                        ve_r[:, :, DK:DK + 1],
                        ones_f32[:].rearrange("p (h x) -> p h x", x=1))

                # ---- Q projection, stripe 0 -----------------------------
                for m in range(2):
                    ps_q = psA.tile([128, QW], F32, tag="psA", name=f"psq{m}")
                    for fc in range(NF):
                        nc.tensor.matmul(
                            ps_q[:],
                            wq_sb[:, fc * GD + m * 128:fc * GD + (m + 1) * 128],
                            XQ[fc][:, 0:QW],
                            start=(fc == 0), stop=(fc == NF - 1))
                    nc.vector.tensor_scalar_add(
                        QTs[0][m][:], ps_q[:], bqv_sb[:, m:m + 1])

            # ---- phase B: striped attention with PE fillers -------------
            with (
                tc.tile_pool(name="ep", bufs=3) as es_pool,
                tc.tile_pool(name="ubp", bufs=8) as ub_pool,
                tc.tile_pool(name="rsp", bufs=2) as rs_pool,
                tc.tile_pool(name="obp", bufs=4) as ob_pool,
                tc.tile_pool(name="psS", bufs=2,
                             space=bass.MemorySpace.PSUM) as psS,
                tc.tile_pool(name="psO", bufs=2,
                             space=bass.MemorySpace.PSUM) as psO,
                tc.tile_pool(name="psF", bufs=2,
                             space=bass.MemorySpace.PSUM) as psF,
            ):
                ub_tiles = {}     # (qs, hp, hh) -> [64, 512] f32 tile
                rs_tiles = {}     # qs -> [128, 512] f32 rowsum-spread tile

                def qproj_fillers(s):
                    fs = []
                    for m in range(2):
                        def mk(mm, fc):
                            def f():
                                ps_q = qproj_fillers.ps[mm]
                                if fc == 0:
                                    ps_q = psF.tile([128, QW], F32, tag="psF",
                                                    name=f"psq{s}_{mm}")
                                    qproj_fillers.ps[mm] = ps_q
                                nc.tensor.matmul(
                                    ps_q[:],
                                    wq_sb[:, fc * GD + mm * 128:
                                          fc * GD + (mm + 1) * 128],
                                    XQ[fc][:, s * QW:(s + 1) * QW],
                                    start=(fc == 0), stop=(fc == NF - 1))
                                if fc == NF - 1:
                                    nc.vector.tensor_scalar_add(
                                        QTs[s][mm][:], ps_q[:],
                                        bqv_sb[:, mm:mm + 1])
                            return f
                        for fc in range(NF):
                            fs.append(mk(m, fc))
                    return fs
                qproj_fillers.ps = [None, None]

                def recip_fillers(s):
                    """Reciprocal dance + normalize for stripe s (rowsums
                    already collected in rows 0..3 of rs_tiles[s])."""
                    fs = []

                    def t_fwd():
                        rsT = rs_pool.tile([128, QW], F32, tag="rsT",
                                           name=f"rsT{s}")
                        nc.vector.transpose(rsT[:], rs_tiles[s][:])
                        nc.scalar.activation(rsT[:], rsT[:], AFT.Ln)
                        rinv = rs_pool.tile([128, QW], BF16, tag="rinv",
                                            name=f"rinv{s}")
                        nc.scalar.activation(rinv[:], rsT[:],
                                             AFT.Exp, scale=-1.0)
                        rinvT = rs_pool.tile([128, QW], BF16, tag="rinvT",
                                             name=f"rinvT{s}")
                        nc.vector.transpose(rinvT[:], rinv[:])
                        recip_fillers.rinvT = rinvT
                    fs.append(t_fwd)

                    def mk_bcast(hp):
                        def f():
                            r_bc = psF.tile([128, QW], F32, tag="psF",
                                            name=f"rbc{s}_{hp}")
                            nc.tensor.matmul(
                                r_bc[:],
                                sel[hp][:],
                                recip_fillers.rinvT[:],
                                start=True, stop=True)
                            for hh in range(2):
                                nc.vector.tensor_mul(
                                    OTs[s][hp][hh * DK:(hh + 1) * DK, :],
                                    ub_tiles.pop((s, hp, hh))[0:DK, :],
                                    r_bc[hh * DK:(hh + 1) * DK, :])
                        return f
                    for hp in range(2):
                        fs.append(mk_bcast(hp))
                    return fs
                recip_fillers.rinvT = None

                def outproj_fillers(s):
                    fs = []

                    def mk(tt, ei):
                        def f():
                            if ei == 0:
                                ob = ob_pool.tile([128, D], BF16, tag="ob",
                                                  name=f"ob{s}_{tt}")
                                outproj_fillers.ob = ob
                            ob = outproj_fillers.ob
                            f_ps = psF.tile([128, QW], F32, tag="psF",
                                            name=f"fps{s}_{tt}_{ei}")
                            for m in range(2):
                                nc.tensor.matmul(
                                    f_ps[:],
                                    OTs[s][m][:, tt * 128:(tt + 1) * 128],
                                    WO[m][:, ei * QW:(ei + 1) * QW],
                                    start=(m == 0), stop=(m == 1))
                            nc.vector.tensor_copy(
                                ob[:, ei * QW:(ei + 1) * QW], f_ps[:])
                            if ei == 1:
                                t0 = (s * 4 + tt) * 128
                                nc.sync.dma_start(out[t0:t0 + 128, :], ob[:])
                        return f
                    for tt in range(4):
                        for ei in range(2):
                            fs.append(mk(tt, ei))
                    return fs
                outproj_fillers.ob = None

                for qs in range(NQS):
                    fillers = deque()
                    if qs > 0:
                        fillers.extend(recip_fillers(qs - 1))
                        fillers.extend(outproj_fillers(qs - 1))
                    if qs < NQS - 1:
                        fillers.extend(qproj_fillers(qs + 1))

                    rs_t = rs_pool.tile([128, QW], F32, tag="rs",
                                        name=f"rs{qs}")
                    nc.gpsimd.memset(rs_t[:], 1.0)
                    rs_tiles[qs] = rs_t

                    for hp in range(2):
                        o_ps = [psO.tile([128, QW], F32, tag="psO",
                                         name=f"o{qs}_{hp}_{i}")
                                for i in range(2)]

                        def emit_av(kt, es):
                            for hh in range(2):
                                h = hp * 2 + hh
                                nc.tensor.matmul(
                                    o_ps[hh][0:DK + 1, :],
                                    VE[kt][:, h * (DK + 1):(h + 1) * (DK + 1)],
                                    es[:, hh * QW:(hh + 1) * QW],
                                    start=(kt == 0), stop=(kt == NKT - 1))

                        # software pipeline: aV(kt-1) is emitted after
                        # scores(kt) so the FIFO PE queue never waits on
                        # exp(kt) before issuing independent scores work.
                        prev_es = None
                        for kt in range(NKT):
                            sc = psS.tile([128, 2 * QW], F32, tag="psS",
                                          name=f"s{qs}_{hp}_{kt}")
                            for hh in range(2):
                                lo = hh * DK
                                nc.tensor.matmul(
                                    sc[:, hh * QW:(hh + 1) * QW],
                                    KT[hp][lo:lo + DK,
                                           kt * 128:(kt + 1) * 128],
                                    QTs[qs][hp][lo:lo + DK, :],
                                    start=True, stop=True)
                            es = es_pool.tile([128, 2 * QW], BF16, tag="es",
                                              name=f"e{qs}_{hp}_{kt}")
                            nc.scalar.activation(es[:], sc[:], AFT.Exp,
                                                 scale=float(SCALE))
                            if prev_es is not None:
                                emit_av(kt - 1, prev_es)
                            prev_es = es
                            if fillers:
                                fillers.popleft()()
                        emit_av(NKT - 1, prev_es)
                        # drain O^T + rowsum row; heads at partitions 32h
                        for hh in range(2):
                            h = hp * 2 + hh
                            u = ub_pool.tile([128, QW], F32, tag="ub",
                                             name=f"ub{qs}_{hp}_{hh}")
                            nc.vector.tensor_copy(u[0:DK, :],
                                                  o_ps[hh][0:DK, :])
                            ub_tiles[(qs, hp, hh)] = u
                            nc.vector.tensor_copy(
                                rs_t[32 * h:32 * h + 1, :],
                                o_ps[hh][DK:DK + 1, :])
                    while fillers:
                        fillers.popleft()()

                # tail: last stripe's reciprocal + output projection
                for f in recip_fillers(NQS - 1):
                    f()
                for f in outproj_fillers(NQS - 1):
                    f()

    from concourse.bacc import get_activation_tables
    import bass_rust as _br
    _combined = "natural_log_exp_and_others"
    _tabs = []
    for _name, _fns in get_activation_tables(nc.m.arch).items():
        if _name != _combined:
            _fns = _fns - {AFT.Exp, AFT.Ln}
        _tabs.append((_name, _fns))
    _br.insert_act_table_loads(nc, _tabs)
    nc.compile()
    return nc


def _numpy_reference(q, k, v, mask, Wq, bq, Wk, bk, Wv, bv, Wo, bo):
    """Fallback for a non-trivial mask (never hit with the stock inputs)."""
    Bn, Tn, _ = q.shape
    H, dk = HEADS, DK

    def split(x):
        return x.reshape(Bn, Tn, H, dk).transpose(0, 2, 1, 3)

    qh = split(q @ Wq + bq)
    kh = split(k @ Wk + bk)
    vh = split(v @ Wv + bv)
    s = np.einsum("bhqd,bhkd->bhqk", qh, kh) / np.sqrt(np.float32(dk))
    s = np.where(mask, s, -np.inf)
    s = s - s.max(axis=-1, keepdims=True)
    e = np.exp(s)
    a = e / e.sum(axis=-1, keepdims=True)
    o = np.einsum("bhqk,bhkd->bhqd", a, vh)
    o = o.transpose(0, 2, 1, 3).reshape(Bn, Tn, H * dk)
    return (o @ Wo + bo).astype(np.float32)


def kernel(q, k, v, mask, Wq, bq, Wk, bk, Wv, bv, Wo, bo):
    global LAST_RESULTS
    q = np.asarray(q, np.float32)
    k = np.asarray(k, np.float32)
    v = np.asarray(v, np.float32)
    mask = np.asarray(mask, bool)
    Wq, bq = np.asarray(Wq, np.float32), np.asarray(bq, np.float32)
    Wk, bk = np.asarray(Wk, np.float32), np.asarray(bk, np.float32)
    Wv, bv = np.asarray(Wv, np.float32), np.asarray(bv, np.float32)
    Wo, bo = np.asarray(Wo, np.float32), np.asarray(bo, np.float32)

    if not mask.all():
        return _numpy_reference(q, k, v, mask, Wq, bq, Wk, bk, Wv, bv, Wo, bo)

    nc = _build_program()

    # host-side sharding
    xT = {}
    for b in range(B):
        xT[b] = tuple(np.ascontiguousarray(x[b].T.astype(BF))
                      for x in (q, k, v))

    def w_chunks(W, g):
        # (1024, 256) head-group slice -> [128, 8*256] chunk-major layout
        Wg = W[:, g * GD:(g + 1) * GD]
        return np.ascontiguousarray(
            Wg.reshape(NF, 128, GD).transpose(1, 0, 2)
            .reshape(128, NF * GD).astype(BF))

    in_maps = []
    for c in range(NCORES):
        b, g = divmod(c, GH)
        xq_t, xk_t, xv_t = xT[b]
        in_maps.append({
            "xqT": xq_t, "xkT": xk_t, "xvT": xv_t,
            "wq": w_chunks(Wq, g), "wk": w_chunks(Wk, g),
            "wv": w_chunks(Wv, g),
            "wo": np.ascontiguousarray(
                Wo[g * GD:(g + 1) * GD, :].astype(BF)).reshape(2, 128, D),
            "bqv": np.ascontiguousarray(
                bq[g * GD:(g + 1) * GD].reshape(2, 128).T),
        })

    LAST_RESULTS = run_bass_kernel_spmd(
        nc, in_maps, list(range(NCORES)),
        trace=bool(os.environ.get("KERNEL_TRACE")))
    res = LAST_RESULTS.results

    const_row = (bv @ Wo + bo).astype(np.float32)  # attn rows sum to 1
    full = np.empty((B, T, D), np.float32)
    for b in range(B):
        acc = res[b * GH]["out"].astype(np.float32)
        for g in range(1, GH):
            acc = acc + res[b * GH + g]["out"].astype(np.float32)
        full[b] = acc + const_row
    return full


# revision 20
# speedup vs baseline: 1.7408x; 1.0367x over previous
"""Trainium2 Bass kernel: 16-head MHA (B=2, T=2048, D=1024, d_k=64).

Sharding (8 NeuronCores): data-parallel over the batch (2) x tensor-parallel
over head groups (4 groups of 4 heads).  Core c handles batch b = c//4 and
heads [4g, 4g+4) with g = c%4.  Each core computes its partial output
    sum_{h in group} softmax((q Wq_h + bq_h)(k Wk_h)^T / 8) (v Wv_h) Wo_h
and the host sums the 4 partials per batch and adds the constant row
bo + bv @ Wo once.  bk is dropped: with the all-ones mask it shifts every
score row by a per-row constant, which softmax ignores exactly.

v2 design notes (vs the 391us baseline):
  * every matmul operand is bf16 (FWL weight loads, fp32 PSUM accumulate);
    output DMA'd as bf16 and upconverted host-side.
  * V is projected directly in [t, v-col] layout (stationary = x^T chunk,
    moving = Wv), killing the 32 PE transposes + drains of v1.
  * attention runs in 512-wide q stripes; per (stripe, head-pair, kt):
    2 scores MMs -> one [128,1024] fp32 PSUM tile, one ACT exp -> bf16,
    2 aV MMs accumulating into per-head [65,512] PSUM.  PSUM budget:
    scores dbuf 2x2 banks + O-accum 2 + filler 2 = 8 banks exactly, so
    scores(kt+1) / exp(kt) / aV(kt) pipeline without stalls.
  * Q projection of stripe s+1, the reciprocal dance, and the output
    projection of stripe s-1 are emitted as PE fillers inside the
    (ACT-bound) attention loop, keeping the PE dense so the HAM clock
    gate stays at 2.4 GHz instead of the baseline's 1.2 GHz cold clock.
  * softmax denominators: rowsums ride along as a 65th V_ext column; the
    reciprocal is computed partition-parallel by DVE 32-block transposing
    the per-head rowsum rows (heads pinned to partitions 0/32/64/96),
    one batched ACT Ln + Exp(-1), transposing back, and broadcasting
    across partitions with per-head rank-1 bf16 matmuls.
"""

import functools
import os
from collections import deque

import ml_dtypes
import numpy as np

import concourse.bass as bass
import concourse.mybir as mybir
import concourse.tile as tile
from concourse import bacc
from concourse.bass_utils import run_bass_kernel_spmd

F32 = mybir.dt.float32
F32R = mybir.dt.float32r
BF16 = mybir.dt.bfloat16
AFT = mybir.ActivationFunctionType
BF = ml_dtypes.bfloat16

D = 1024          # model dim
T = 2048          # sequence length
B = 2             # batch
HEADS = 16        # total heads
DK = 64           # head dim
NCORES = 8
GH = 4            # heads per core
GD = GH * DK      # 256 projection cols per core
NF = D // 128     # 8 contraction chunks
NKT = T // 128    # 16 k tiles
NQS = 4           # 512-wide q stripes
QW = T // NQS     # 512
SCALE = 1.0 / np.sqrt(np.float32(DK))  # 1/8

# Results of the last run (for test harness introspection: exec_time_ns etc.)
LAST_RESULTS = None


@functools.lru_cache(maxsize=1)
def _build_program():
    nc = bacc.Bacc("TRN2", target_bir_lowering=False, debug=False,
                   num_devices=NCORES)

    xqT = nc.declare_dram_parameter("xqT", [D, T], BF16, isOutput=False)
    xkT = nc.declare_dram_parameter("xkT", [D, T], BF16, isOutput=False)
    xvT = nc.declare_dram_parameter("xvT", [D, T], BF16, isOutput=False)
    wq = nc.declare_dram_parameter("wq", [128, NF * GD], BF16, isOutput=False)
    wk = nc.declare_dram_parameter("wk", [128, NF * GD], BF16, isOutput=False)
    wv = nc.declare_dram_parameter("wv", [128, NF * GD], BF16, isOutput=False)
    wo = nc.declare_dram_parameter("wo", [2, 128, D], BF16, isOutput=False)
    bqv = nc.declare_dram_parameter("bqv", [128, 2], F32, isOutput=False)
    out = nc.declare_dram_parameter("out", [T, D], BF16, isOutput=True)

    with tile.TileContext(nc) as tc:
        # ---- persistent pools -------------------------------------------
        with (
            tc.tile_pool(name="kt", bufs=2) as kt_pool,
            tc.tile_pool(name="vext", bufs=NKT) as vext_pool,
            tc.tile_pool(name="qts", bufs=NQS * 2) as qts_pool,
            tc.tile_pool(name="ots", bufs=NQS * 2) as ots_pool,
            tc.tile_pool(name="wts", bufs=3) as w_pool,
            tc.tile_pool(name="wop", bufs=2) as wo_pool,
            tc.tile_pool(name="xq", bufs=NF) as xq_pool,
            tc.tile_pool(name="const", bufs=1) as const_pool,
        ):
            ones_f32 = const_pool.tile([128, GH], F32, tag="ones32")
            nc.gpsimd.memset(ones_f32[:], 1.0)
            ones_bf = const_pool.tile([128, DK], BF16, tag="onesbf")
            nc.gpsimd.memset(ones_bf[:], 1.0)
            # head-pair selectors: sel[hp][c, m] = (c == 32*(hp*2 + m//64)),
            # i.e. r_bc[m, :] = rinvT[32*head(m), :] after the C=128 matmul
            sel = [const_pool.tile([128, 128], BF16, tag=f"sel{hp}",
                                   name=f"sel{hp}") for hp in range(2)]
            for hp in range(2):
                nc.gpsimd.memset(sel[hp][:], 0.0)
                for hh in range(2):
                    c = 32 * (hp * 2 + hh)
                    nc.vector.tensor_copy(
                        sel[hp][c:c + 1, hh * DK:(hh + 1) * DK],
                        ones_bf[0:1, 0:DK])
            bqv_sb = const_pool.tile([128, 2], F32, tag="bqv")
            nc.sync.dma_start(bqv_sb[:], bqv[:])

            KT = [kt_pool.tile([128, T], BF16, tag="kt", name=f"kt{m}")
                  for m in range(2)]
            VE = [vext_pool.tile([128, GH * (DK + 1)], BF16, tag="vext",
                                 name=f"ve{i}") for i in range(NKT)]
            # per-stripe Q^T and O^T tiles (heads of pair hp stacked 64+64)
            QTs = [[qts_pool.tile([128, QW], BF16, tag="qts",
                                  name=f"qt{s}_{m}") for m in range(2)]
                   for s in range(NQS)]
            OTs = [[ots_pool.tile([128, QW], BF16, tag="ots",
                                  name=f"ot{s}_{m}") for m in range(2)]
                   for s in range(NQS)]
            WO = [wo_pool.tile([128, D], BF16, tag="wop", name=f"wo{m}")
                  for m in range(2)]

            wq_sb = w_pool.tile([128, NF * GD], BF16, tag="w", name="wq_sb")
            wk_sb = w_pool.tile([128, NF * GD], BF16, tag="w", name="wk_sb")
            wv_sb = w_pool.tile([128, NF * GD], BF16, tag="w", name="wv_sb")
            XQ = [xq_pool.tile([128, T], BF16, tag="xq", name=f"xq{i}")
                  for i in range(NF)]

            # ---- DMA in: wq + first q-stripe first (Q0 proj warms the
            # PE), then K and V (attention gate), then the q remainder.
            nc.sync.dma_start(wq_sb[:], wq[:])
            for fc in range(NF):
                nc.sync.dma_start(XQ[fc][:, 0:QW],
                                  xqT[fc * 128:(fc + 1) * 128, 0:QW])
            nc.sync.dma_start(wk_sb[:], wk[:])
            xk_tiles = []
            xv_tiles = []
            with (
                tc.tile_pool(name="xkv", bufs=2 * NF) as xkv_pool,
                tc.tile_pool(name="psA", bufs=8,
                             space=bass.MemorySpace.PSUM) as psA,
            ):
                for fc in range(NF):
                    xk_t = xkv_pool.tile([128, T], BF16, tag="xkv",
                                         name=f"xk{fc}")
                    nc.sync.dma_start(xk_t[:], xkT[fc * 128:(fc + 1) * 128, :])
                    xk_tiles.append(xk_t)
                nc.sync.dma_start(wv_sb[:], wv[:])
                for fc in range(NF):
                    xv_t = xkv_pool.tile([128, T], BF16, tag="xkv",
                                         name=f"xv{fc}")
                    nc.sync.dma_start(xv_t[:], xvT[fc * 128:(fc + 1) * 128, :])
                    xv_tiles.append(xv_t)
                for fc in range(NF):
                    nc.sync.dma_start(XQ[fc][:, QW:T],
                                      xqT[fc * 128:(fc + 1) * 128, QW:T])
                nc.sync.dma_start(WO[0][:], wo[0])
                nc.sync.dma_start(WO[1][:], wo[1])

                # ---- Q projection, stripe 0 (first: warms the PE) -------
                for m in range(2):
                    ps_q = psA.tile([128, QW], F32, tag="psA", name=f"psq{m}")
                    for fc in range(NF):
                        nc.tensor.matmul(
                            ps_q[:],
                            wq_sb[:, fc * GD + m * 128:fc * GD + (m + 1) * 128],
                            XQ[fc][:, 0:QW],
                            start=(fc == 0), stop=(fc == NF - 1))
                    nc.vector.tensor_scalar_add(
                        QTs[0][m][:], ps_q[:], bqv_sb[:, m:m + 1])

                # ---- K projection (transposed layout) -------------------
                ps_k = [psA.tile([128, QW], F32, tag="psA", name=f"psk{i}")
                        for i in range(8)]
                for fc in range(NF):
                    for m in range(2):
                        for qh in range(4):
                            nc.tensor.matmul(
                                ps_k[m * 4 + qh][:],
                                wk_sb[:, fc * GD + m * 128:
                                      fc * GD + (m + 1) * 128],
                                xk_tiles[fc][:, qh * QW:(qh + 1) * QW],
                                start=(fc == 0), stop=(fc == NF - 1))
                for m in range(2):
                    for qh in range(4):
                        nc.vector.tensor_copy(
                            KT[m][:, qh * QW:(qh + 1) * QW],
                            ps_k[m * 4 + qh][:])

                # ---- V projection, direct [t, vcol] layout --------------
                for tb in range(NKT):
                    ps_v = psA.tile([128, QW], F32, tag="psA", name=f"psv{tb}")
                    for dc in range(NF):
                        nc.tensor.matmul(
                            ps_v[:, 0:GD],
                            xv_tiles[dc][:, tb * 128:(tb + 1) * 128],
                            wv_sb[:, dc * GD:(dc + 1) * GD],
                            start=(dc == 0), stop=(dc == NF - 1))
                    ve_r = VE[tb][:].rearrange("p (h x) -> p h x", x=DK + 1)
                    nc.vector.tensor_copy(
                        ve_r[:, :, 0:DK],
                        ps_v[:, 0:GD].rearrange("p (h x) -> p h x", x=DK))
                    nc.vector.tensor_copy(
                        ve_r[:, :, DK:DK + 1],
                        ones_f32[:].rearrange("p (h x) -> p h x", x=1))

            # ---- phase B: striped attention with PE fillers -------------
            with (
                tc.tile_pool(name="ep", bufs=3) as es_pool,
                tc.tile_pool(name="ubp", bufs=8) as ub_pool,
                tc.tile_pool(name="rsp", bufs=2) as rs_pool,
                tc.tile_pool(name="obp", bufs=4) as ob_pool,
                tc.tile_pool(name="psS", bufs=2,
                             space=bass.MemorySpace.PSUM) as psS,
                tc.tile_pool(name="psO", bufs=2,
                             space=bass.MemorySpace.PSUM) as psO,
                tc.tile_pool(name="psF", bufs=2,
                             space=bass.MemorySpace.PSUM) as psF,
            ):
                ub_tiles = {}     # (qs, hp, hh) -> [64, 512] f32 tile
                rs_tiles = {}     # qs -> [128, 512] f32 rowsum-spread tile

                def qproj_fillers(s):
                    fs = []
                    for m in range(2):
                        def mk(mm, fc):
                            def f():
                                ps_q = qproj_fillers.ps[mm]
                                if fc == 0:
                                    ps_q = psF.tile([128, QW], F32, tag="psF",
                                                    name=f"psq{s}_{mm}")
                                    qproj_fillers.ps[mm] = ps_q
                                nc.tensor.matmul(
                                    ps_q[:],
                                    wq_sb[:, fc * GD + mm * 128:
                                          fc * GD + (mm + 1) * 128],
                                    XQ[fc][:, s * QW:(s + 1) * QW],
                                    start=(fc == 0), stop=(fc == NF - 1))
                                if fc == NF - 1:
                                    nc.vector.tensor_scalar_add(
                                        QTs[s][mm][:], ps_q[:],
                                        bqv_sb[:, mm:mm + 1])
                            return f
                        for fc in range(NF):
                            fs.append(mk(m, fc))
                    return fs
                qproj_fillers.ps = [None, None]

                def recip_fillers(s, hps=(0, 1), state={}):
                    """Reciprocal dance + normalize for stripe s (rowsums
                    already collected at partitions 32h of rs_tiles[s]).
                    Split into [transpose, ln/exp/transpose, bcast...] so
                    the caller can space the ACT work away from its DVE
                    dependency in the filler stream."""
                    fs = []

                    def t1():
                        rsT = rs_pool.tile([128, QW], F32, tag="rsT",
                                           name=f"rsT{s}")
                        nc.vector.transpose(rsT[:], rs_tiles[s][:])
                        state[s] = rsT
                    fs.append(t1)

                    def t2():
                        rsT = state.pop(s)
                        nc.scalar.activation(rsT[:], rsT[:], AFT.Ln)
                        rinv = rs_pool.tile([128, QW], BF16, tag="rinv",
                                            name=f"rinv{s}")
                        nc.scalar.activation(rinv[:], rsT[:],
                                             AFT.Exp, scale=-1.0)
                        rinvT = rs_pool.tile([128, QW], BF16, tag="rinvT",
                                             name=f"rinvT{s}")
                        nc.vector.transpose(rinvT[:], rinv[:])
                        recip_fillers.rinvT = rinvT
                    fs.append(t2)

                    def mk_bcast(hp):
                        def f():
                            r_bc = psF.tile([128, QW], F32, tag="psF",
                                            name=f"rbc{s}_{hp}")
                            nc.tensor.matmul(
                                r_bc[:],
                                sel[hp][:],
                                recip_fillers.rinvT[:],
                                start=True, stop=True)
                            for hh in range(2):
                                nc.vector.tensor_mul(
                                    OTs[s][hp][hh * DK:(hh + 1) * DK, :],
                                    ub_tiles.pop((s, hp, hh))[0:DK, :],
                                    r_bc[hh * DK:(hh + 1) * DK, :])
                        return f
                    for hp in hps:
                        fs.append(mk_bcast(hp))
                    return fs
                recip_fillers.rinvT = None

                def outproj_fillers(s):
                    fs = []

                    def mk(tt, ei):
                        def f():
                            if ei == 0:
                                ob = ob_pool.tile([128, D], BF16, tag="ob",
                                                  name=f"ob{s}_{tt}")
                                outproj_fillers.ob = ob
                            ob = outproj_fillers.ob
                            f_ps = psF.tile([128, QW], F32, tag="psF",
                                            name=f"fps{s}_{tt}_{ei}")
                            for m in range(2):
                                nc.tensor.matmul(
                                    f_ps[:],
                                    OTs[s][m][:, tt * 128:(tt + 1) * 128],
                                    WO[m][:, ei * QW:(ei + 1) * QW],
                                    start=(m == 0), stop=(m == 1))
                            nc.vector.tensor_copy(
                                ob[:, ei * QW:(ei + 1) * QW], f_ps[:])
                            if ei == 1:
                                t0 = (s * 4 + tt) * 128
                                nc.sync.dma_start(out[t0:t0 + 128, :], ob[:])
                        return f
                    for tt in range(4):
                        for ei in range(2):
                            fs.append(mk(tt, ei))
                    return fs
                outproj_fillers.ob = None

                # flat (qs, hp, kt) stream: aV(step-1) is emitted after
                # scores/exp(step) so the FIFO PE queue never waits on an
                # exp before issuing independent scores work, including
                # across block and stripe boundaries.
                fillers = deque()
                pending = [None]  # (qs, hp, o_ps, es, kt)

                def flush_pending():
                    p = pending[0]
                    if p is None:
                        return
                    pending[0] = None
                    pqs, php, po_ps, pes, pkt = p
                    for hh in range(2):
                        h = php * 2 + hh
                        nc.tensor.matmul(
                            po_ps[hh][0:DK + 1, :],
                            VE[pkt][:, h * (DK + 1):(h + 1) * (DK + 1)],
                            pes[:, hh * QW:(hh + 1) * QW],
                            start=(pkt == 0), stop=(pkt == NKT - 1))
                    if pkt == NKT - 1:
                        # drain O^T + rowsum row; heads at partitions 32h
                        for hh in range(2):
                            h = php * 2 + hh
                            u = ub_pool.tile([128, QW], F32, tag="ub",
                                             name=f"ub{pqs}_{php}_{hh}")
                            nc.vector.tensor_copy(u[0:DK, :],
                                                  po_ps[hh][0:DK, :])
                            ub_tiles[(pqs, php, hh)] = u
                            nc.vector.tensor_copy(
                                rs_tiles[pqs][32 * h:32 * h + 1, :],
                                po_ps[hh][DK:DK + 1, :])
                        if pqs == NQS - 1 and php == 0:
                            # last stripe: overlap hp0's half of the
                            # reciprocal dance under hp1's attention
                            rf = recip_fillers(pqs, hps=(0,))
                            fillers.append(rf[0])
                            fillers.extend([spacer] * 3)
                            fillers.extend(rf[1:])

                def spacer():
                    pass

                for qs in range(NQS):
                    rf = recip_fillers(qs - 1) if qs > 0 else []
                    qp = qproj_fillers(qs + 1) if qs < NQS - 1 else []
                    if rf:
                        fillers.append(rf[0])       # DVE transpose
                        if qp:
                            fillers.extend(qp[0:8])  # qproj m0 (pins psF)
                        else:
                            fillers.extend([spacer] * 4)
                        fillers.extend(rf[1:])      # Ln/Exp + bcasts
                        fillers.extend(outproj_fillers(qs - 1))
                        fillers.extend(qp[8:16])    # qproj m1
                    else:
                        fillers.extend(qp)

                    rs_t = rs_pool.tile([128, QW], F32, tag="rs",
                                        name=f"rs{qs}")
                    nc.gpsimd.memset(rs_t[:], 1.0)
                    rs_tiles[qs] = rs_t

                    for hp in range(2):
                        o_ps = [psO.tile([128, QW], F32, tag="psO",
                                         name=f"o{qs}_{hp}_{i}")
                                for i in range(2)]
                        for kt in range(NKT):
                            sc = psS.tile([128, 2 * QW], F32, tag="psS",
                                          name=f"s{qs}_{hp}_{kt}")
                            for hh in range(2):
                                lo = hh * DK
                                nc.tensor.matmul(
                                    sc[:, hh * QW:(hh + 1) * QW],
                                    KT[hp][lo:lo + DK,
                                           kt * 128:(kt + 1) * 128],
                                    QTs[qs][hp][lo:lo + DK, :],
                                    start=True, stop=True)
                            es = es_pool.tile([128, 2 * QW], BF16, tag="es",
                                              name=f"e{qs}_{hp}_{kt}")
                            nc.scalar.activation(es[:], sc[:], AFT.Exp,
                                                 scale=float(SCALE))
                            flush_pending()
                            pending[0] = (qs, hp, o_ps, es, kt)
                            if fillers:
                                fillers.popleft()()
                    # leftover fillers must land before the next stripe's
                    # scores read tiles they write (QTs of qs+1)
                    while fillers:
                        fillers.popleft()()

                # tail: flush last aV + drains, hp1 dance, outproj
                flush_pending()
                for f in recip_fillers(NQS - 1, hps=(1,)):
                    f()
                for f in outproj_fillers(NQS - 1):
                    f()

    from concourse.bacc import get_activation_tables
    import bass_rust as _br
    _combined = "natural_log_exp_and_others"
    _tabs = []
    for _name, _fns in get_activation_tables(nc.m.arch).items():
        if _name != _combined:
            _fns = _fns - {AFT.Exp, AFT.Ln}
        _tabs.append((_name, _fns))
    _br.insert_act_table_loads(nc, _tabs)
    nc.compile()
    return nc


def _numpy_reference(q, k, v, mask, Wq, bq, Wk, bk, Wv, bv, Wo, bo):
    """Fallback for a non-trivial mask (never hit with the stock inputs)."""
    Bn, Tn, _ = q.shape
    H, dk = HEADS, DK

    def split(x):
        return x.reshape(Bn, Tn, H, dk).transpose(0, 2, 1, 3)

    qh = split(q @ Wq + bq)
    kh = split(k @ Wk + bk)
    vh = split(v @ Wv + bv)
    s = np.einsum("bhqd,bhkd->bhqk", qh, kh) / np.sqrt(np.float32(dk))
    s = np.where(mask, s, -np.inf)
    s = s - s.max(axis=-1, keepdims=True)
    e = np.exp(s)
    a = e / e.sum(axis=-1, keepdims=True)
    o = np.einsum("bhqk,bhkd->bhqd", a, vh)
    o = o.transpose(0, 2, 1, 3).reshape(Bn, Tn, H * dk)
    return (o @ Wo + bo).astype(np.float32)


def kernel(q, k, v, mask, Wq, bq, Wk, bk, Wv, bv, Wo, bo):
    global LAST_RESULTS
    q = np.asarray(q, np.float32)
    k = np.asarray(k, np.float32)
    v = np.asarray(v, np.float32)
    mask = np.asarray(mask, bool)
    Wq, bq = np.asarray(Wq, np.float32), np.asarray(bq, np.float32)
    Wk, bk = np.asarray(Wk, np.float32), np.asarray(bk, np.float32)
    Wv, bv = np.asarray(Wv, np.float32), np.asarray(bv, np.float32)
    Wo, bo = np.asarray(Wo, np.float32), np.asarray(bo, np.float32)

    if not mask.all():
        return _numpy_reference(q, k, v, mask, Wq, bq, Wk, bk, Wv, bv, Wo, bo)

    nc = _build_program()

    # host-side sharding
    xT = {}
    for b in range(B):
        xT[b] = tuple(np.ascontiguousarray(x[b].T.astype(BF))
                      for x in (q, k, v))

    def w_chunks(W, g):
        # (1024, 256) head-group slice -> [128, 8*256] chunk-major layout
        Wg = W[:, g * GD:(g + 1) * GD]
        return np.ascontiguousarray(
            Wg.reshape(NF, 128, GD).transpose(1, 0, 2)
            .reshape(128, NF * GD).astype(BF))

    in_maps = []
    for c in range(NCORES):
        b, g = divmod(c, GH)
        xq_t, xk_t, xv_t = xT[b]
        in_maps.append({
            "xqT": xq_t, "xkT": xk_t, "xvT": xv_t,
            "wq": w_chunks(Wq, g), "wk": w_chunks(Wk, g),
            "wv": w_chunks(Wv, g),
            "wo": np.ascontiguousarray(
                Wo[g * GD:(g + 1) * GD, :].astype(BF)).reshape(2, 128, D),
            "bqv": np.ascontiguousarray(
                bq[g * GD:(g + 1) * GD].reshape(2, 128).T),
        })

    LAST_RESULTS = run_bass_kernel_spmd(
        nc, in_maps, list(range(NCORES)),
        trace=bool(os.environ.get("KERNEL_TRACE")))
    res = LAST_RESULTS.results

    const_row = (bv @ Wo + bo).astype(np.float32)  # attn rows sum to 1
    full = np.empty((B, T, D), np.float32)
    for b in range(B):
        acc = res[b * GH]["out"].astype(np.float32)
        for g in range(1, GH):
            acc = acc + res[b * GH + g]["out"].astype(np.float32)
        full[b] = acc + const_row
    return full


# revision 32
# speedup vs baseline: 1.8093x; 1.0394x over previous
"""Trainium2 Bass kernel: 16-head MHA (B=2, T=2048, D=1024, d_k=64).

Sharding (8 NeuronCores): data-parallel over the batch (2) x tensor-parallel
over head groups (4 groups of 4 heads).  Core c handles batch b = c//4 and
heads [4g, 4g+4) with g = c%4.  Each core computes its partial output
    sum_{h in group} softmax((q Wq_h + bq_h)(k Wk_h)^T / 8) (v Wv_h) Wo_h
and the host sums the 4 partials per batch and adds the constant row
bo + bv @ Wo once.  bk is dropped: with the all-ones mask it shifts every
score row by a per-row constant, which softmax ignores exactly.

v2 design notes (vs the 391us baseline):
  * every matmul operand is bf16 (FWL weight loads, fp32 PSUM accumulate);
    output DMA'd as bf16 and upconverted host-side.
  * V is projected directly in [t, v-col] layout (stationary = x^T chunk,
    moving = Wv), killing the 32 PE transposes + drains of v1.
  * attention runs in 512-wide q stripes; per (stripe, head-pair, kt):
    2 scores MMs -> one [128,1024] fp32 PSUM tile, one ACT exp -> bf16,
    2 aV MMs accumulating into per-head [65,512] PSUM.  PSUM budget:
    scores dbuf 2x2 banks + O-accum 2 + filler 2 = 8 banks exactly, so
    scores(kt+1) / exp(kt) / aV(kt) pipeline without stalls.
  * Q projection of stripe s+1, the reciprocal dance, and the output
    projection of stripe s-1 are emitted as PE fillers inside the
    (ACT-bound) attention loop, keeping the PE dense so the HAM clock
    gate stays at 2.4 GHz instead of the baseline's 1.2 GHz cold clock.
  * softmax denominators: rowsums ride along as a 65th V_ext column; the
    reciprocal is computed partition-parallel by DVE 32-block transposing
    the per-head rowsum rows (heads pinned to partitions 0/32/64/96),
    one batched ACT Ln + Exp(-1), transposing back, and broadcasting
    across partitions with per-head rank-1 bf16 matmuls.
"""

import functools
import os
from collections import deque

import ml_dtypes
import numpy as np

import concourse.bass as bass
import concourse.mybir as mybir
import concourse.tile as tile
from concourse import bacc
from concourse.bass_utils import run_bass_kernel_spmd

F32 = mybir.dt.float32
F32R = mybir.dt.float32r
BF16 = mybir.dt.bfloat16
AFT = mybir.ActivationFunctionType
BF = ml_dtypes.bfloat16

D = 1024          # model dim
T = 2048          # sequence length
B = 2             # batch
HEADS = 16        # total heads
DK = 64           # head dim
NCORES = 8
GH = 4            # heads per core
GD = GH * DK      # 256 projection cols per core
NF = D // 128     # 8 contraction chunks
NKT = T // 128    # 16 k tiles
NQS = 4           # 512-wide q stripes
QW = T // NQS     # 512
SCALE = 1.0 / np.sqrt(np.float32(DK))  # 1/8

# Results of the last run (for test harness introspection: exec_time_ns etc.)
LAST_RESULTS = None


@functools.lru_cache(maxsize=1)
def _build_program():
    nc = bacc.Bacc("TRN2", target_bir_lowering=False, debug=False,
                   num_devices=NCORES)

    # host-packed activation layouts (see _pack_* in kernel()):
    #   xq[s]  = [128, NF*QW]  q-stripe s, chunk-major (8 KiB DMA lines)
    #   xk[qh] = [128, NF*QW]  k column-group qh, chunk-major
    #   xv[tb] = [128, NF*128] k-tile tb, chunk-major (2 KiB lines)
    xq = nc.declare_dram_parameter("xq", [NQS, 128, NF * QW], BF16,
                                   isOutput=False)
    xk = nc.declare_dram_parameter("xk", [4, 128, NF * QW], BF16,
                                   isOutput=False)
    xv = nc.declare_dram_parameter("xv", [NKT, 128, NF * 128], BF16,
                                   isOutput=False)
    wq = nc.declare_dram_parameter("wq", [128, NF * GD], BF16, isOutput=False)
    wk = nc.declare_dram_parameter("wk", [128, NF * GD], BF16, isOutput=False)
    wv = nc.declare_dram_parameter("wv", [128, NF * GD], BF16, isOutput=False)
    wo = nc.declare_dram_parameter("wo", [2, 128, D], BF16, isOutput=False)
    bqv = nc.declare_dram_parameter("bqv", [128, 2], F32, isOutput=False)
    out = nc.declare_dram_parameter("out", [T, D], BF16, isOutput=True)

    import contextlib
    with tile.TileContext(nc) as tc, contextlib.ExitStack() as _st:
        # ---- persistent pools -------------------------------------------
        def _pool(**kw):
            return _st.enter_context(tc.tile_pool(**kw))

        if True:
            kt_pool = _pool(name="kt", bufs=2)
            vext_pool = _pool(name="vext", bufs=NKT)
            qts_pool = _pool(name="qts", bufs=NQS * 2)
            ots_pool = _pool(name="ots", bufs=NQS * 2)
            w_pool = _pool(name="wts", bufs=3)
            wo_pool = _pool(name="wop", bufs=2)
            xq_pool = _pool(name="xq", bufs=NQS)
            xk_pool = _pool(name="xk", bufs=4)
            xv_pool = _pool(name="xv", bufs=NKT)
            const_pool = _pool(name="const", bufs=1)
            ones_f32 = const_pool.tile([128, GH], F32, tag="ones32")
            nc.gpsimd.memset(ones_f32[:], 1.0)
            ones_bf = const_pool.tile([128, DK], BF16, tag="onesbf")
            nc.gpsimd.memset(ones_bf[:], 1.0)
            # head-pair selectors: sel[hp][c, m] = (c == 32*(hp*2 + m//64)),
            # i.e. r_bc[m, :] = rinvT[32*head(m), :] after the C=128 matmul
            sel = [const_pool.tile([128, 128], BF16, tag=f"sel{hp}",
                                   name=f"sel{hp}") for hp in range(2)]
            for hp in range(2):
                nc.gpsimd.memset(sel[hp][:], 0.0)
                for hh in range(2):
                    c = 32 * (hp * 2 + hh)
                    nc.vector.tensor_copy(
                        sel[hp][c:c + 1, hh * DK:(hh + 1) * DK],
                        ones_bf[0:1, 0:DK])
            bqv_sb = const_pool.tile([128, 2], F32, tag="bqv")
            nc.sync.dma_start(bqv_sb[:], bqv[:])

            KT = [kt_pool.tile([128, T], BF16, tag="kt", name=f"kt{m}")
                  for m in range(2)]
            VE = [vext_pool.tile([128, GH * (DK + 1)], BF16, tag="vext",
                                 name=f"ve{i}") for i in range(NKT)]
            # per-stripe Q^T and O^T tiles (heads of pair hp stacked 64+64)
            QTs = [[qts_pool.tile([128, QW], BF16, tag="qts",
                                  name=f"qt{s}_{m}") for m in range(2)]
                   for s in range(NQS)]
            OTs = [[ots_pool.tile([128, QW], BF16, tag="ots",
                                  name=f"ot{s}_{m}") for m in range(2)]
                   for s in range(NQS)]
            WO = [wo_pool.tile([128, D], BF16, tag="wop", name=f"wo{m}")
                  for m in range(2)]

            wq_sb = w_pool.tile([128, NF * GD], BF16, tag="w", name="wq_sb")
            wk_sb = w_pool.tile([128, NF * GD], BF16, tag="w", name="wk_sb")
            wv_sb = w_pool.tile([128, NF * GD], BF16, tag="w", name="wv_sb")
            XQs = [xq_pool.tile([128, NF * QW], BF16, tag="xq",
                                name=f"xqs{s}") for s in range(NQS)]
            XKq = [xk_pool.tile([128, NF * QW], BF16, tag="xk",
                                name=f"xkq{i}") for i in range(4)]
            XVt = [xv_pool.tile([128, NF * 128], BF16, tag="xv",
                                name=f"xvt{i}") for i in range(NKT)]

            # V_ext ones columns (persistent; written once, no DMA dep)
            for tb in range(NKT):
                ve_r = VE[tb][:].rearrange("p (h x) -> p h x", x=DK + 1)
                nc.vector.tensor_copy(
                    ve_r[:, :, DK:DK + 1],
                    ones_f32[:].rearrange("p (h x) -> p h x", x=1))

            # ---- DMA in, ordered so attention can start after K groups
            # 0-2 and V tiles 0-11 land; the rest arrives under stripe 0.
            nc.sync.dma_start(wq_sb[:], wq[:])
            nc.sync.dma_start(XQs[0][:], xq[0])
            nc.sync.dma_start(wk_sb[:], wk[:])
            for qh in range(3):
                nc.sync.dma_start(XKq[qh][:], xk[qh])
            nc.sync.dma_start(wv_sb[:], wv[:])
            for tb in range(12):
                nc.sync.dma_start(XVt[tb][:], xv[tb])
            nc.sync.dma_start(XKq[3][:], xk[3])
            for tb in range(12, NKT):
                nc.sync.dma_start(XVt[tb][:], xv[tb])
            for s in range(1, NQS):
                nc.sync.dma_start(XQs[s][:], xq[s])
            nc.sync.dma_start(WO[0][:], wo[0])
            nc.sync.dma_start(WO[1][:], wo[1])

            def q_project(s, m, ps_q, fc):
                nc.tensor.matmul(
                    ps_q[:],
                    wq_sb[:, fc * GD + m * 128:fc * GD + (m + 1) * 128],
                    XQs[s][:, fc * QW:(fc + 1) * QW],
                    start=(fc == 0), stop=(fc == NF - 1))
                if fc == NF - 1:
                    nc.vector.tensor_scalar_add(
                        QTs[s][m][:], ps_q[:], bqv_sb[:, m:m + 1])

            def k_project(qh, m, ps_k, fc):
                nc.tensor.matmul(
                    ps_k[:],
                    wk_sb[:, fc * GD + m * 128:fc * GD + (m + 1) * 128],
                    XKq[qh][:, fc * QW:(fc + 1) * QW],
                    start=(fc == 0), stop=(fc == NF - 1))
                if fc == NF - 1:
                    nc.vector.tensor_copy(
                        KT[m][:, qh * QW:(qh + 1) * QW], ps_k[:])

            def v_project(tb, ps_v, dc):
                nc.tensor.matmul(
                    ps_v[:, 0:GD],
                    XVt[tb][:, dc * 128:(dc + 1) * 128],
                    wv_sb[:, dc * GD:(dc + 1) * GD],
                    start=(dc == 0), stop=(dc == NF - 1))
                if dc == NF - 1:
                    ve_r = VE[tb][:].rearrange("p (h x) -> p h x", x=DK + 1)
                    nc.vector.tensor_copy(
                        ve_r[:, :, 0:DK],
                        ps_v[:, 0:GD].rearrange("p (h x) -> p h x", x=DK))

            # ---- phase A: Q stripe 0, K groups 0-2, V tiles 0-11 --------
            with tc.tile_pool(name="psA", bufs=8,
                              space=bass.MemorySpace.PSUM) as psA:
                for m in range(2):
                    ps_q = psA.tile([128, QW], F32, tag="psA", name=f"psq{m}")
                    for fc in range(NF):
                        q_project(0, m, ps_q, fc)
                for qh in range(3):
                    for m in range(2):
                        ps_k = psA.tile([128, QW], F32, tag="psA",
                                        name=f"psk{qh}_{m}")
                        for fc in range(NF):
                            k_project(qh, m, ps_k, fc)
                for tb in range(12):
                    ps_v = psA.tile([128, QW], F32, tag="psA",
                                    name=f"psv{tb}")
                    for dc in range(NF):
                        v_project(tb, ps_v, dc)

            # ---- phase B: striped attention with PE fillers -------------
            with contextlib.ExitStack() as _stB:
                def _poolB(**kw):
                    return _stB.enter_context(tc.tile_pool(**kw))

                es_pool = _poolB(name="ep", bufs=4)
                ub_pool = _poolB(name="ubp", bufs=8)
                rs_pool = _poolB(name="rsp", bufs=2)
                ob_pool = _poolB(name="obp", bufs=4)
                psS = _poolB(name="psS", bufs=2,
                             space=bass.MemorySpace.PSUM)
                psO = _poolB(name="psO", bufs=2,
                             space=bass.MemorySpace.PSUM)
                psF = _poolB(name="psF", bufs=2,
                             space=bass.MemorySpace.PSUM)
                ub_tiles = {}     # (qs, hp, hh) -> [64, 512] f32 tile
                rs_tiles = {}     # qs -> [128, 512] f32 rowsum-spread tile

                fstate = {}

                def qproj_fillers(s):
                    fs = []
                    for m in range(2):
                        def mk(mm, fc):
                            def f():
                                if fc == 0:
                                    fstate['q', mm] = psF.tile(
                                        [128, QW], F32, tag="psF",
                                        name=f"psq{s}_{mm}")
                                q_project(s, mm, fstate['q', mm], fc)
                            return f
                        for fc in range(NF):
                            fs.append(mk(m, fc))
                    return fs

                def kq3_fillers():
                    """K projection of column-group 3 (2 MMs per filler)."""
                    fs = []
                    for m in range(2):
                        def mk(mm, fp):
                            def f():
                                if fp == 0:
                                    fstate['k', mm] = psF.tile(
                                        [128, QW], F32, tag="psF",
                                        name=f"psk3_{mm}")
                                for fc in (2 * fp, 2 * fp + 1):
                                    k_project(3, mm, fstate['k', mm], fc)
                            return f
                        for fp in range(4):
                            fs.append(mk(m, fp))
                    return fs

                def vtb_fillers(tb):
                    """V projection of k-tile tb (4 MMs per filler)."""
                    def mk(dp):
                        def f():
                            if dp == 0:
                                fstate['v', tb] = psF.tile(
                                    [128, QW], F32, tag="psF",
                                    name=f"psv{tb}")
                            for dc in range(4 * dp, 4 * dp + 4):
                                v_project(tb, fstate['v', tb], dc)
                        return f
                    return [mk(0), mk(1)]

                def recip_fillers(s, hps=(0, 1), state={}):
                    """Reciprocal dance + normalize for stripe s (rowsums
                    already collected at partitions 32h of rs_tiles[s]).
                    Split into [transpose, ln/exp/transpose, bcast...] so
                    the caller can space the ACT work away from its DVE
                    dependency in the filler stream."""
                    fs = []

                    def t1():
                        rsT = rs_pool.tile([128, QW], F32, tag="rsT",
                                           name=f"rsT{s}")
                        nc.vector.transpose(rsT[:], rs_tiles[s][:])
                        state[s] = rsT
                    fs.append(t1)

                    def t2():
                        rsT = state.pop(s)
                        nc.scalar.activation(rsT[:], rsT[:], AFT.Ln)
                        rinv = rs_pool.tile([128, QW], BF16, tag="rinv",
                                            name=f"rinv{s}")
                        nc.scalar.activation(rinv[:], rsT[:],
                                             AFT.Exp, scale=-1.0)
                        rinvT = rs_pool.tile([128, QW], BF16, tag="rinvT",
                                             name=f"rinvT{s}")
                        nc.vector.transpose(rinvT[:], rinv[:])
                        recip_fillers.rinvT = rinvT
                    fs.append(t2)

                    def mk_bcast(hp):
                        def f():
                            r_bc = psF.tile([128, QW], F32, tag="psF",
                                            name=f"rbc{s}_{hp}")
                            nc.tensor.matmul(
                                r_bc[:],
                                sel[hp][:],
                                recip_fillers.rinvT[:],
                                start=True, stop=True)
                            for hh in range(2):
                                nc.vector.tensor_mul(
                                    OTs[s][hp][hh * DK:(hh + 1) * DK, :],
                                    ub_tiles.pop((s, hp, hh))[0:DK, :],
                                    r_bc[hh * DK:(hh + 1) * DK, :])
                        return f
                    for hp in hps:
                        fs.append(mk_bcast(hp))
                    return fs
                recip_fillers.rinvT = None

                def outproj_fillers(s):
                    fs = []

                    def mk(tt, ei):
                        def f():
                            if ei == 0:
                                ob = ob_pool.tile([128, D], BF16, tag="ob",
                                                  name=f"ob{s}_{tt}")
                                outproj_fillers.ob = ob
                            ob = outproj_fillers.ob
                            f_ps = psF.tile([128, QW], F32, tag="psF",
                                            name=f"fps{s}_{tt}_{ei}")
                            for m in range(2):
                                nc.tensor.matmul(
                                    f_ps[:],
                                    OTs[s][m][:, tt * 128:(tt + 1) * 128],
                                    WO[m][:, ei * QW:(ei + 1) * QW],
                                    start=(m == 0), stop=(m == 1))
                            nc.vector.tensor_copy(
                                ob[:, ei * QW:(ei + 1) * QW], f_ps[:])
                            if ei == 1:
                                t0 = (s * 4 + tt) * 128
                                nc.sync.dma_start(out[t0:t0 + 128, :], ob[:])
                        return f
                    for tt in range(4):
                        for ei in range(2):
                            fs.append(mk(tt, ei))
                    return fs
                outproj_fillers.ob = None

                # flat (qs, hp, kt) stream: aV is emitted 1-3 steps behind
                # scores/exp so the FIFO PE queue never waits on an exp
                # before issuing independent scores work.  At block starts
                # the hold-back deepens to 3 so the previous block's DVE
                # drains (which gate aV(kt0) via o_ps buffer reuse) finish
                # under the run-ahead scores instead of stalling the PE.
                fillers = deque()
                pending = deque()  # (qs, hp, o_ps, es, kt)

                def flush_one():
                    pqs, php, po_ps, pes, pkt = pending.popleft()
                    for hh in range(2):
                        h = php * 2 + hh
                        nc.tensor.matmul(
                            po_ps[hh][0:DK + 1, :],
                            VE[pkt][:, h * (DK + 1):(h + 1) * (DK + 1)],
                            pes[:, hh * QW:(hh + 1) * QW],
                            start=(pkt == 0), stop=(pkt == NKT - 1))
                    if pkt == NKT - 1:
                        # drain O^T + rowsum row; heads at partitions 32h.
                        # The very last block's drains go on the otherwise
                        # idle ACT queue to shorten the serial tail.
                        last = pqs == NQS - 1 and php == 1
                        for hh in range(2):
                            h = php * 2 + hh
                            u = ub_pool.tile([128, QW], F32, tag="ub",
                                             name=f"ub{pqs}_{php}_{hh}")
                            if last and hh == 1:
                                nc.scalar.activation(
                                    u[0:DK, :], po_ps[hh][0:DK, :],
                                    AFT.Copy)
                            else:
                                nc.vector.tensor_copy(
                                    u[0:DK, :], po_ps[hh][0:DK, :])
                            ub_tiles[(pqs, php, hh)] = u
                            nc.vector.tensor_copy(
                                rs_tiles[pqs][32 * h:32 * h + 1, :],
                                po_ps[hh][DK:DK + 1, :])
                        if pqs == NQS - 1 and php == 0:
                            # last stripe: overlap hp0's half of the
                            # reciprocal dance under hp1's attention
                            rf = recip_fillers(pqs, hps=(0,))
                            fillers.append(rf[0])
                            fillers.extend([spacer] * 3)
                            fillers.extend(rf[1:])

                def spacer():
                    pass

                for qs in range(NQS):
                    rf = recip_fillers(qs - 1) if qs > 0 else []
                    qp = qproj_fillers(qs + 1) if qs < NQS - 1 else []
                    if qs == 0:
                        # remaining input projections ride along stripe 0:
                        # K group 3 (8), V tiles 12-15 (8), then qproj(1)
                        fillers.extend(kq3_fillers())
                        for tb in range(12, NKT):
                            fillers.extend(vtb_fillers(tb))
                        fillers.extend(qp)
                    elif rf:
                        fillers.append(rf[0])       # DVE transpose
                        if qp:
                            fillers.extend(qp[0:8])  # qproj m0 (pins psF)
                        else:
                            fillers.extend([spacer] * 4)
                        fillers.extend(rf[1:])      # Ln/Exp + bcasts
                        fillers.extend(outproj_fillers(qs - 1))
                        fillers.extend(qp[8:16])    # qproj m1
                    else:
                        fillers.extend(qp)

                    rs_t = rs_pool.tile([128, QW], F32, tag="rs",
                                        name=f"rs{qs}")
                    nc.gpsimd.memset(rs_t[:], 1.0)
                    rs_tiles[qs] = rs_t

                    for hp in range(2):
                        o_ps = [psO.tile([128, QW], F32, tag="psO",
                                         name=f"o{qs}_{hp}_{i}")
                                for i in range(2)]
                        for kt in range(NKT):
                            sc = psS.tile([128, 2 * QW], F32, tag="psS",
                                          name=f"s{qs}_{hp}_{kt}")
                            for hh in range(2):
                                lo = hh * DK
                                nc.tensor.matmul(
                                    sc[:, hh * QW:(hh + 1) * QW],
                                    KT[hp][lo:lo + DK,
                                           kt * 128:(kt + 1) * 128],
                                    QTs[qs][hp][lo:lo + DK, :],
                                    start=True, stop=True)
                            es = es_pool.tile([128, 2 * QW], BF16, tag="es",
                                              name=f"e{qs}_{hp}_{kt}")
                            nc.scalar.activation(es[:], sc[:], AFT.Exp,
                                                 scale=float(SCALE))
                            # flush older blocks now; hold up to 3 of the
                            # current block while kt < 3
                            while pending and pending[0][0:2] != (qs, hp):
                                flush_one()
                            pending.append((qs, hp, o_ps, es, kt))
                            target = 3 if kt < 3 else 1
                            while len(pending) > target:
                                flush_one()
                            if fillers:
                                fillers.popleft()()
                    # leftover fillers must land before the next stripe's
                    # scores read tiles they write (QTs of qs+1)
                    while fillers:
                        fillers.popleft()()

                # tail: flush last aV + drains, hp1 dance, outproj
                while pending:
                    flush_one()
                for f in recip_fillers(NQS - 1, hps=(1,)):
                    f()
                for f in outproj_fillers(NQS - 1):
                    f()

    from concourse.bacc import get_activation_tables
    import bass_rust as _br
    _combined = "natural_log_exp_and_others"
    _tabs = []
    for _name, _fns in get_activation_tables(nc.m.arch).items():
        if _name != _combined:
            _fns = _fns - {AFT.Exp, AFT.Ln}
        _tabs.append((_name, _fns))
    _br.insert_act_table_loads(nc, _tabs)
    nc.compile()
    return nc


def _numpy_reference(q, k, v, mask, Wq, bq, Wk, bk, Wv, bv, Wo, bo):
    """Fallback for a non-trivial mask (never hit with the stock inputs)."""
    Bn, Tn, _ = q.shape
    H, dk = HEADS, DK

    def split(x):
        return x.reshape(Bn, Tn, H, dk).transpose(0, 2, 1, 3)

    qh = split(q @ Wq + bq)
    kh = split(k @ Wk + bk)
    vh = split(v @ Wv + bv)
    s = np.einsum("bhqd,bhkd->bhqk", qh, kh) / np.sqrt(np.float32(dk))
    s = np.where(mask, s, -np.inf)
    s = s - s.max(axis=-1, keepdims=True)
    e = np.exp(s)
    a = e / e.sum(axis=-1, keepdims=True)
    o = np.einsum("bhqk,bhkd->bhqd", a, vh)
    o = o.transpose(0, 2, 1, 3).reshape(Bn, Tn, H * dk)
    return (o @ Wo + bo).astype(np.float32)


def kernel(q, k, v, mask, Wq, bq, Wk, bk, Wv, bv, Wo, bo):
    global LAST_RESULTS
    q = np.asarray(q, np.float32)
    k = np.asarray(k, np.float32)
    v = np.asarray(v, np.float32)
    mask = np.asarray(mask, bool)
    Wq, bq = np.asarray(Wq, np.float32), np.asarray(bq, np.float32)
    Wk, bk = np.asarray(Wk, np.float32), np.asarray(bk, np.float32)
    Wv, bv = np.asarray(Wv, np.float32), np.asarray(bv, np.float32)
    Wo, bo = np.asarray(Wo, np.float32), np.asarray(bo, np.float32)

    if not mask.all():
        return _numpy_reference(q, k, v, mask, Wq, bq, Wk, bk, Wv, bv, Wo, bo)

    nc = _build_program()

    # host-side sharding; activations packed chunk-major per column
    # group (see the dram parameter comments in _build_program)
    def pack_cols(xT_b, w):
        ng = T // w
        return np.ascontiguousarray(
            xT_b.reshape(NF, 128, ng, w).transpose(2, 1, 0, 3)
            .reshape(ng, 128, NF * w))

    xP = {}
    for b in range(B):
        xq_t, xk_t, xv_t = (x[b].T.astype(BF) for x in (q, k, v))
        xP[b] = (pack_cols(xq_t, QW), pack_cols(xk_t, QW),
                 pack_cols(xv_t, 128))

    def w_chunks(W, g):
        # (1024, 256) head-group slice -> [128, 8*256] chunk-major layout
        Wg = W[:, g * GD:(g + 1) * GD]
        return np.ascontiguousarray(
            Wg.reshape(NF, 128, GD).transpose(1, 0, 2)
            .reshape(128, NF * GD).astype(BF))

    in_maps = []
    for c in range(NCORES):
        b, g = divmod(c, GH)
        xq_t, xk_t, xv_t = xP[b]
        in_maps.append({
            "xq": xq_t, "xk": xk_t, "xv": xv_t,
            "wq": w_chunks(Wq, g), "wk": w_chunks(Wk, g),
            "wv": w_chunks(Wv, g),
            "wo": np.ascontiguousarray(
                Wo[g * GD:(g + 1) * GD, :].astype(BF)).reshape(2, 128, D),
            "bqv": np.ascontiguousarray(
                bq[g * GD:(g + 1) * GD].reshape(2, 128).T),
        })

    LAST_RESULTS = run_bass_kernel_spmd(
        nc, in_maps, list(range(NCORES)),
        trace=bool(os.environ.get("KERNEL_TRACE")))
    res = LAST_RESULTS.results

    const_row = (bv @ Wo + bo).astype(np.float32)  # attn rows sum to 1
    full = np.empty((B, T, D), np.float32)
    for b in range(B):
        acc = res[b * GH]["out"].astype(np.float32)
        for g in range(1, GH):
            acc = acc + res[b * GH + g]["out"].astype(np.float32)
        full[b] = acc + const_row
    return full


# revision 39
# speedup vs baseline: 1.8162x; 1.0038x over previous
"""Trainium2 Bass kernel: 16-head MHA (B=2, T=2048, D=1024, d_k=64).

Sharding (8 NeuronCores): data-parallel over the batch (2) x tensor-parallel
over head groups (4 groups of 4 heads).  Core c handles batch b = c//4 and
heads [4g, 4g+4) with g = c%4.  Each core computes its partial output
    sum_{h in group} softmax((q Wq_h + bq_h)(k Wk_h)^T / 8) (v Wv_h) Wo_h
and the host sums the 4 partials per batch and adds the constant row
bo + bv @ Wo once.  bk is dropped: with the all-ones mask it shifts every
score row by a per-row constant, which softmax ignores exactly.

v2 design notes (vs the 391us baseline):
  * every matmul operand is bf16 (FWL weight loads, fp32 PSUM accumulate);
    output DMA'd as bf16 and upconverted host-side.
  * V is projected directly in [t, v-col] layout (stationary = x^T chunk,
    moving = Wv), killing the 32 PE transposes + drains of v1.
  * attention runs in 512-wide q stripes; per (stripe, head-pair, kt):
    2 scores MMs -> one [128,1024] fp32 PSUM tile, one ACT exp -> bf16,
    2 aV MMs accumulating into per-head [65,512] PSUM.  PSUM budget:
    scores dbuf 2x2 banks + O-accum 2 + filler 2 = 8 banks exactly, so
    scores(kt+1) / exp(kt) / aV(kt) pipeline without stalls.
  * Q projection of stripe s+1, the reciprocal dance, and the output
    projection of stripe s-1 are emitted as PE fillers inside the
    (ACT-bound) attention loop, keeping the PE dense so the HAM clock
    gate stays at 2.4 GHz instead of the baseline's 1.2 GHz cold clock.
  * softmax denominators: rowsums ride along as a 65th V_ext column; the
    reciprocal is computed partition-parallel by DVE 32-block transposing
    the per-head rowsum rows (heads pinned to partitions 0/32/64/96),
    one batched ACT Ln + Exp(-1), transposing back, and broadcasting
    across partitions with per-head rank-1 bf16 matmuls.
"""

import functools
import os
from collections import deque

import ml_dtypes
import numpy as np

import concourse.bass as bass
import concourse.mybir as mybir
import concourse.tile as tile
from concourse import bacc
from concourse.bass_utils import run_bass_kernel_spmd

F32 = mybir.dt.float32
F32R = mybir.dt.float32r
BF16 = mybir.dt.bfloat16
AFT = mybir.ActivationFunctionType
BF = ml_dtypes.bfloat16

D = 1024          # model dim
T = 2048          # sequence length
B = 2             # batch
HEADS = 16        # total heads
DK = 64           # head dim
NCORES = 8
GH = 4            # heads per core
GD = GH * DK      # 256 projection cols per core
NF = D // 128     # 8 contraction chunks
NKT = T // 128    # 16 k tiles
NQS = 4           # 512-wide q stripes
QW = T // NQS     # 512
SCALE = 1.0 / np.sqrt(np.float32(DK))  # 1/8

# Results of the last run (for test harness introspection: exec_time_ns etc.)
LAST_RESULTS = None


@functools.lru_cache(maxsize=1)
def _build_program():
    nc = bacc.Bacc("TRN2", target_bir_lowering=False, debug=False,
                   num_devices=NCORES)

    # host-packed activation layouts (see _pack_* in kernel()):
    #   xq[s]  = [128, NF*QW]  q-stripe s, chunk-major (8 KiB DMA lines)
    #   xk[qh] = [128, NF*QW]  k column-group qh, chunk-major
    #   xv[tb] = [128, NF*128] k-tile tb, chunk-major (2 KiB lines)
    xq = nc.declare_dram_parameter("xq", [NQS, 128, NF * QW], BF16,
                                   isOutput=False)
    xk = nc.declare_dram_parameter("xk", [4, 128, NF * QW], BF16,
                                   isOutput=False)
    xv = nc.declare_dram_parameter("xv", [NKT, 128, NF * 128], BF16,
                                   isOutput=False)
    wq = nc.declare_dram_parameter("wq", [128, NF * GD], BF16, isOutput=False)
    wk = nc.declare_dram_parameter("wk", [128, NF * GD], BF16, isOutput=False)
    wv = nc.declare_dram_parameter("wv", [128, NF * GD], BF16, isOutput=False)
    wo = nc.declare_dram_parameter("wo", [2, 128, D], BF16, isOutput=False)
    bqv = nc.declare_dram_parameter("bqv", [128, 2], F32, isOutput=False)
    out = nc.declare_dram_parameter("out", [T, D], BF16, isOutput=True)

    import contextlib
    with tile.TileContext(nc) as tc, contextlib.ExitStack() as _st:
        # ---- persistent pools -------------------------------------------
        def _pool(**kw):
            return _st.enter_context(tc.tile_pool(**kw))

        if True:
            kt_pool = _pool(name="kt", bufs=2)
            vext_pool = _pool(name="vext", bufs=NKT)
            qts_pool = _pool(name="qts", bufs=NQS * 2)
            ots_pool = _pool(name="ots", bufs=NQS * 2)
            w_pool = _pool(name="wts", bufs=3)
            wo_pool = _pool(name="wop", bufs=2)
            xq_pool = _pool(name="xq", bufs=NQS)
            xk_pool = _pool(name="xk", bufs=4)
            xv_pool = _pool(name="xv", bufs=NKT)
            const_pool = _pool(name="const", bufs=1)
            ones_f32 = const_pool.tile([128, GH], F32, tag="ones32")
            nc.gpsimd.memset(ones_f32[:], 1.0)
            ones_bf = const_pool.tile([128, DK], BF16, tag="onesbf")
            nc.gpsimd.memset(ones_bf[:], 1.0)
            # head-pair selectors: sel[hp][c, m] = (c == 32*(hp*2 + m//64)),
            # i.e. r_bc[m, :] = rinvT[32*head(m), :] after the C=128 matmul
            sel = [const_pool.tile([128, 128], BF16, tag=f"sel{hp}",
                                   name=f"sel{hp}") for hp in range(2)]
            for hp in range(2):
                nc.gpsimd.memset(sel[hp][:], 0.0)
                for hh in range(2):
                    c = 32 * (hp * 2 + hh)
                    nc.vector.tensor_copy(
                        sel[hp][c:c + 1, hh * DK:(hh + 1) * DK],
                        ones_bf[0:1, 0:DK])
            bqv_sb = const_pool.tile([128, 2], F32, tag="bqv")
            nc.sync.dma_start(bqv_sb[:], bqv[:])

            KT = [kt_pool.tile([128, T], BF16, tag="kt", name=f"kt{m}")
                  for m in range(2)]
            VE = [vext_pool.tile([128, GH * (DK + 1)], BF16, tag="vext",
                                 name=f"ve{i}") for i in range(NKT)]
            # per-stripe Q^T and O^T tiles (heads of pair hp stacked 64+64)
            QTs = [[qts_pool.tile([128, QW], BF16, tag="qts",
                                  name=f"qt{s}_{m}") for m in range(2)]
                   for s in range(NQS)]
            OTs = [[ots_pool.tile([128, QW], BF16, tag="ots",
                                  name=f"ot{s}_{m}") for m in range(2)]
                   for s in range(NQS)]
            WO = [wo_pool.tile([128, D], BF16, tag="wop", name=f"wo{m}")
                  for m in range(2)]

            wq_sb = w_pool.tile([128, NF * GD], BF16, tag="w", name="wq_sb")
            wk_sb = w_pool.tile([128, NF * GD], BF16, tag="w", name="wk_sb")
            wv_sb = w_pool.tile([128, NF * GD], BF16, tag="w", name="wv_sb")
            XQs = [xq_pool.tile([128, NF * QW], BF16, tag="xq",
                                name=f"xqs{s}") for s in range(NQS)]
            XKq = [xk_pool.tile([128, NF * QW], BF16, tag="xk",
                                name=f"xkq{i}") for i in range(4)]
            XVt = [xv_pool.tile([128, NF * 128], BF16, tag="xv",
                                name=f"xvt{i}") for i in range(NKT)]

            # V_ext ones columns (persistent; written once, no DMA dep)
            for tb in range(NKT):
                ve_r = VE[tb][:].rearrange("p (h x) -> p h x", x=DK + 1)
                nc.vector.tensor_copy(
                    ve_r[:, :, DK:DK + 1],
                    ones_f32[:].rearrange("p (h x) -> p h x", x=1))

            # ---- DMA in, ordered to match the projection filler schedule
            # (upfront: Q stripe 0, K group 0, V tiles 0-5; the rest lands
            # under stripe 0's attention).
            nc.sync.dma_start(wq_sb[:], wq[:])
            nc.sync.dma_start(XQs[0][:], xq[0])
            nc.sync.dma_start(wk_sb[:], wk[:])
            nc.sync.dma_start(XKq[0][:], xk[0])
            nc.sync.dma_start(wv_sb[:], wv[:])
            for tb in range(6):
                nc.sync.dma_start(XVt[tb][:], xv[tb])
            nc.sync.dma_start(XKq[1][:], xk[1])
            for tb in range(6, 10):
                nc.sync.dma_start(XVt[tb][:], xv[tb])
            nc.sync.dma_start(XKq[2][:], xk[2])
            nc.sync.dma_start(XKq[3][:], xk[3])
            for tb in range(10, NKT):
                nc.sync.dma_start(XVt[tb][:], xv[tb])
            for s in range(1, NQS):
                nc.sync.dma_start(XQs[s][:], xq[s])
            nc.sync.dma_start(WO[0][:], wo[0])
            nc.sync.dma_start(WO[1][:], wo[1])

            def q_project(s, m, ps_q, fc):
                nc.tensor.matmul(
                    ps_q[:],
                    wq_sb[:, fc * GD + m * 128:fc * GD + (m + 1) * 128],
                    XQs[s][:, fc * QW:(fc + 1) * QW],
                    start=(fc == 0), stop=(fc == NF - 1))
                if fc == NF - 1:
                    nc.vector.tensor_scalar_add(
                        QTs[s][m][:], ps_q[:], bqv_sb[:, m:m + 1])

            def k_project(qh, m, ps_k, fc):
                nc.tensor.matmul(
                    ps_k[:],
                    wk_sb[:, fc * GD + m * 128:fc * GD + (m + 1) * 128],
                    XKq[qh][:, fc * QW:(fc + 1) * QW],
                    start=(fc == 0), stop=(fc == NF - 1))
                if fc == NF - 1:
                    nc.vector.tensor_copy(
                        KT[m][:, qh * QW:(qh + 1) * QW], ps_k[:])

            def v_project(tb, ps_v, dc):
                nc.tensor.matmul(
                    ps_v[:, 0:GD],
                    XVt[tb][:, dc * 128:(dc + 1) * 128],
                    wv_sb[:, dc * GD:(dc + 1) * GD],
                    start=(dc == 0), stop=(dc == NF - 1))
                if dc == NF - 1:
                    ve_r = VE[tb][:].rearrange("p (h x) -> p h x", x=DK + 1)
                    nc.vector.tensor_copy(
                        ve_r[:, :, 0:DK],
                        ps_v[:, 0:GD].rearrange("p (h x) -> p h x", x=DK))

            # ---- phase A: warm the HAM clock gate with dummy matmuls
            # (no DMA dependency), then project Q stripe 0, K group 0 and
            # V tiles 0-5 as their inputs land.
            with tc.tile_pool(name="psA", bufs=8,
                              space=bass.MemorySpace.PSUM) as psA:
                warm = psA.tile([128, QW], F32, tag="psA", name="warm")
                for i in range(48):
                    nc.tensor.matmul(
                        warm[0:DK, 0:DK], ones_bf[:, 0:DK],
                        ones_bf[:, 0:DK], start=True, stop=True)
                for m in range(2):
                    ps_q = psA.tile([128, QW], F32, tag="psA", name=f"psq{m}")
                    for fc in range(NF):
                        q_project(0, m, ps_q, fc)
                for m in range(2):
                    ps_k = psA.tile([128, QW], F32, tag="psA",
                                    name=f"psk0_{m}")
                    for fc in range(NF):
                        k_project(0, m, ps_k, fc)
                for tb in range(6):
                    ps_v = psA.tile([128, QW], F32, tag="psA",
                                    name=f"psv{tb}")
                    for dc in range(NF):
                        v_project(tb, ps_v, dc)

            # ---- phase B: striped attention with PE fillers -------------
            with contextlib.ExitStack() as _stB:
                def _poolB(**kw):
                    return _stB.enter_context(tc.tile_pool(**kw))

                es_pool = _poolB(name="ep", bufs=4)
                ub_pool = _poolB(name="ubp", bufs=8)
                rs_pool = _poolB(name="rsp", bufs=2)
                ob_pool = _poolB(name="obp", bufs=4)
                psS = _poolB(name="psS", bufs=2,
                             space=bass.MemorySpace.PSUM)
                psO = _poolB(name="psO", bufs=2,
                             space=bass.MemorySpace.PSUM)
                psF = _poolB(name="psF", bufs=2,
                             space=bass.MemorySpace.PSUM)
                ub_tiles = {}     # (qs, hp, hh) -> [64, 512] f32 tile
                rs_tiles = {}     # qs -> [128, 512] f32 rowsum-spread tile

                fstate = {}

                def qproj_fillers(s):
                    fs = []
                    for m in range(2):
                        def mk(mm, fc):
                            def f():
                                if fc == 0:
                                    fstate['q', mm] = psF.tile(
                                        [128, QW], F32, tag="psF",
                                        name=f"psq{s}_{mm}")
                                q_project(s, mm, fstate['q', mm], fc)
                            return f
                        for fc in range(NF):
                            fs.append(mk(m, fc))
                    return fs

                def kq_fillers(qh, m):
                    """K projection of column-group qh, head-pair tile m
                    (2 MMs per filler)."""
                    def mk(fp):
                        def f():
                            if fp == 0:
                                fstate['k', qh, m] = psF.tile(
                                    [128, QW], F32, tag="psF",
                                    name=f"psk{qh}_{m}")
                            for fc in (2 * fp, 2 * fp + 1):
                                k_project(qh, m, fstate['k', qh, m], fc)
                        return f
                    return [mk(fp) for fp in range(4)]

                def vtb_fillers(tb):
                    """V projection of k-tile tb (4 MMs per filler)."""
                    def mk(dp):
                        def f():
                            if dp == 0:
                                fstate['v', tb] = psF.tile(
                                    [128, QW], F32, tag="psF",
                                    name=f"psv{tb}")
                            for dc in range(4 * dp, 4 * dp + 4):
                                v_project(tb, fstate['v', tb], dc)
                        return f
                    return [mk(0), mk(1)]

                def recip_fillers(s, hps=(0, 1), state={}):
                    """Reciprocal dance + normalize for stripe s (rowsums
                    already collected at partitions 32h of rs_tiles[s]).
                    Split into [transpose, ln/exp/transpose, bcast...] so
                    the caller can space the ACT work away from its DVE
                    dependency in the filler stream."""
                    fs = []

                    def t1():
                        rsT = rs_pool.tile([128, QW], F32, tag="rsT",
                                           name=f"rsT{s}")
                        nc.vector.transpose(rsT[:], rs_tiles[s][:])
                        state[s] = rsT
                    fs.append(t1)

                    def t2():
                        rsT = state.pop(s)
                        nc.scalar.activation(rsT[:], rsT[:], AFT.Ln)
                        rinv = rs_pool.tile([128, QW], BF16, tag="rinv",
                                            name=f"rinv{s}")
                        nc.scalar.activation(rinv[:], rsT[:],
                                             AFT.Exp, scale=-1.0)
                        rinvT = rs_pool.tile([128, QW], BF16, tag="rinvT",
                                             name=f"rinvT{s}")
                        nc.vector.transpose(rinvT[:], rinv[:])
                        recip_fillers.rinvT = rinvT
                    fs.append(t2)

                    def mk_bcast(hp):
                        def f():
                            r_bc = psF.tile([128, QW], F32, tag="psF",
                                            name=f"rbc{s}_{hp}")
                            nc.tensor.matmul(
                                r_bc[:],
                                sel[hp][:],
                                recip_fillers.rinvT[:],
                                start=True, stop=True)
                            for hh in range(2):
                                nc.vector.tensor_mul(
                                    OTs[s][hp][hh * DK:(hh + 1) * DK, :],
                                    ub_tiles.pop((s, hp, hh))[0:DK, :],
                                    r_bc[hh * DK:(hh + 1) * DK, :])
                        return f
                    for hp in hps:
                        fs.append(mk_bcast(hp))
                    return fs
                recip_fillers.rinvT = None

                def outproj_fillers(s):
                    fs = []

                    def mk(tt, ei):
                        def f():
                            if ei == 0:
                                ob = ob_pool.tile([128, D], BF16, tag="ob",
                                                  name=f"ob{s}_{tt}")
                                outproj_fillers.ob = ob
                            ob = outproj_fillers.ob
                            f_ps = psF.tile([128, QW], F32, tag="psF",
                                            name=f"fps{s}_{tt}_{ei}")
                            for m in range(2):
                                nc.tensor.matmul(
                                    f_ps[:],
                                    OTs[s][m][:, tt * 128:(tt + 1) * 128],
                                    WO[m][:, ei * QW:(ei + 1) * QW],
                                    start=(m == 0), stop=(m == 1))
                            if s == NQS - 1:
                                # tail: ACT is idle, DVE is the tail chain
                                nc.scalar.activation(
                                    ob[:, ei * QW:(ei + 1) * QW], f_ps[:],
                                    AFT.Copy)
                            else:
                                nc.vector.tensor_copy(
                                    ob[:, ei * QW:(ei + 1) * QW], f_ps[:])
                            if ei == 1:
                                t0 = (s * 4 + tt) * 128
                                nc.sync.dma_start(out[t0:t0 + 128, :], ob[:])
                        return f
                    for tt in range(4):
                        for ei in range(2):
                            fs.append(mk(tt, ei))
                    return fs
                outproj_fillers.ob = None

                # flat (qs, hp, kt) stream: aV is emitted 1-3 steps behind
                # scores/exp so the FIFO PE queue never waits on an exp
                # before issuing independent scores work.  At block starts
                # the hold-back deepens to 3 so the previous block's DVE
                # drains (which gate aV(kt0) via o_ps buffer reuse) finish
                # under the run-ahead scores instead of stalling the PE.
                fillers = deque()
                pending = deque()  # (qs, hp, o_ps, es, kt)

                def flush_one():
                    pqs, php, po_ps, pes, pkt = pending.popleft()
                    for hh in range(2):
                        h = php * 2 + hh
                        nc.tensor.matmul(
                            po_ps[hh][0:DK + 1, :],
                            VE[pkt][:, h * (DK + 1):(h + 1) * (DK + 1)],
                            pes[:, hh * QW:(hh + 1) * QW],
                            start=(pkt == 0), stop=(pkt == NKT - 1))
                    if pkt == NKT - 1:
                        # drain O^T + rowsum row; heads at partitions 32h.
                        # The very last block's drains go on the otherwise
                        # idle ACT queue to shorten the serial tail.
                        last = pqs == NQS - 1 and php == 1
                        for hh in range(2):
                            h = php * 2 + hh
                            u = ub_pool.tile([128, QW], F32, tag="ub",
                                             name=f"ub{pqs}_{php}_{hh}")
                            if last and hh == 1:
                                nc.scalar.activation(
                                    u[0:DK, :], po_ps[hh][0:DK, :],
                                    AFT.Copy)
                            else:
                                nc.vector.tensor_copy(
                                    u[0:DK, :], po_ps[hh][0:DK, :])
                            ub_tiles[(pqs, php, hh)] = u
                            nc.vector.tensor_copy(
                                rs_tiles[pqs][32 * h:32 * h + 1, :],
                                po_ps[hh][DK:DK + 1, :])
                        if pqs == NQS - 1 and php == 0:
                            # last stripe: overlap hp0's half of the
                            # reciprocal dance under hp1's attention
                            rf = recip_fillers(pqs, hps=(0,))
                            fillers.append(rf[0])
                            fillers.extend([spacer] * 3)
                            fillers.extend(rf[1:])

                def spacer():
                    pass

                for qs in range(NQS):
                    rf = recip_fillers(qs - 1) if qs > 0 else []
                    qp = qproj_fillers(qs + 1) if qs < NQS - 1 else []
                    if qs == 0:
                        # remaining input projections ride along stripe 0
                        # (2 filler pops per kt), ordered so every tile's
                        # drain is emitted before its first consumer
                        fillers.extend(kq_fillers(1, 0))
                        for tb in range(6, 10):
                            fillers.extend(vtb_fillers(tb))
                        fillers.extend(kq_fillers(2, 0))
                        fillers.extend(kq_fillers(3, 0))
                        for tb in range(10, NKT):
                            fillers.extend(vtb_fillers(tb))
                        for qh in range(1, 4):
                            fillers.extend(kq_fillers(qh, 1))
                        fillers.extend(qp)
                    elif rf:
                        fillers.append(rf[0])       # DVE transpose
                        if qp:
                            fillers.extend(qp[0:8])  # qproj m0 (pins psF)
                        else:
                            fillers.extend([spacer] * 4)
                        fillers.extend(rf[1:])      # Ln/Exp + bcasts
                        fillers.extend(outproj_fillers(qs - 1))
                        fillers.extend(qp[8:16])    # qproj m1
                    else:
                        fillers.extend(qp)

                    rs_t = rs_pool.tile([128, QW], F32, tag="rs",
                                        name=f"rs{qs}")
                    nc.gpsimd.memset(rs_t[:], 1.0)
                    rs_tiles[qs] = rs_t

                    for hp in range(2):
                        o_ps = [psO.tile([128, QW], F32, tag="psO",
                                         name=f"o{qs}_{hp}_{i}")
                                for i in range(2)]
                        for kt in range(NKT):
                            sc = psS.tile([128, 2 * QW], F32, tag="psS",
                                          name=f"s{qs}_{hp}_{kt}")
                            for hh in range(2):
                                lo = hh * DK
                                nc.tensor.matmul(
                                    sc[:, hh * QW:(hh + 1) * QW],
                                    KT[hp][lo:lo + DK,
                                           kt * 128:(kt + 1) * 128],
                                    QTs[qs][hp][lo:lo + DK, :],
                                    start=True, stop=True)
                            es = es_pool.tile([128, 2 * QW], BF16, tag="es",
                                              name=f"e{qs}_{hp}_{kt}")
                            nc.scalar.activation(es[:], sc[:], AFT.Exp,
                                                 scale=float(SCALE))
                            # flush older blocks now; hold up to 3 of the
                            # current block while kt < 3
                            while pending and pending[0][0:2] != (qs, hp):
                                flush_one()
                            pending.append((qs, hp, o_ps, es, kt))
                            target = 3 if kt < 3 else 1
                            while len(pending) > target:
                                flush_one()
                            for _ in range(2 if qs == 0 else 1):
                                if fillers:
                                    fillers.popleft()()
                    # leftover fillers must land before the next stripe's
                    # scores read tiles they write (QTs of qs+1)
                    while fillers:
                        fillers.popleft()()

                # tail: flush last aV + drains, hp1 dance, outproj
                while pending:
                    flush_one()
                for f in recip_fillers(NQS - 1, hps=(1,)):
                    f()
                for f in outproj_fillers(NQS - 1):
                    f()

    from concourse.bacc import get_activation_tables
    import bass_rust as _br
    _combined = "natural_log_exp_and_others"
    _tabs = []
    for _name, _fns in get_activation_tables(nc.m.arch).items():
        if _name != _combined:
            _fns = _fns - {AFT.Exp, AFT.Ln}
        _tabs.append((_name, _fns))
    _br.insert_act_table_loads(nc, _tabs)
    nc.compile()
    return nc


def _numpy_reference(q, k, v, mask, Wq, bq, Wk, bk, Wv, bv, Wo, bo):
    """Fallback for a non-trivial mask (never hit with the stock inputs)."""
    Bn, Tn, _ = q.shape
    H, dk = HEADS, DK

    def split(x):
        return x.reshape(Bn, Tn, H, dk).transpose(0, 2, 1, 3)

    qh = split(q @ Wq + bq)
    kh = split(k @ Wk + bk)
    vh = split(v @ Wv + bv)
    s = np.einsum("bhqd,bhkd->bhqk", qh, kh) / np.sqrt(np.float32(dk))
    s = np.where(mask, s, -np.inf)
    s = s - s.max(axis=-1, keepdims=True)
    e = np.exp(s)
    a = e / e.sum(axis=-1, keepdims=True)
    o = np.einsum("bhqk,bhkd->bhqd", a, vh)
    o = o.transpose(0, 2, 1, 3).reshape(Bn, Tn, H * dk)
    return (o @ Wo + bo).astype(np.float32)


def kernel(q, k, v, mask, Wq, bq, Wk, bk, Wv, bv, Wo, bo):
    global LAST_RESULTS
    q = np.asarray(q, np.float32)
    k = np.asarray(k, np.float32)
    v = np.asarray(v, np.float32)
    mask = np.asarray(mask, bool)
    Wq, bq = np.asarray(Wq, np.float32), np.asarray(bq, np.float32)
    Wk, bk = np.asarray(Wk, np.float32), np.asarray(bk, np.float32)
    Wv, bv = np.asarray(Wv, np.float32), np.asarray(bv, np.float32)
    Wo, bo = np.asarray(Wo, np.float32), np.asarray(bo, np.float32)

    if not mask.all():
        return _numpy_reference(q, k, v, mask, Wq, bq, Wk, bk, Wv, bv, Wo, bo)

    nc = _build_program()

    # host-side sharding; activations packed chunk-major per column
    # group (see the dram parameter comments in _build_program)
    def pack_cols(xT_b, w):
        ng = T // w
        return np.ascontiguousarray(
            xT_b.reshape(NF, 128, ng, w).transpose(2, 1, 0, 3)
            .reshape(ng, 128, NF * w))

    xP = {}
    for b in range(B):
        xq_t, xk_t, xv_t = (x[b].T.astype(BF) for x in (q, k, v))
        xP[b] = (pack_cols(xq_t, QW), pack_cols(xk_t, QW),
                 pack_cols(xv_t, 128))

    def w_chunks(W, g):
        # (1024, 256) head-group slice -> [128, 8*256] chunk-major layout
        Wg = W[:, g * GD:(g + 1) * GD]
        return np.ascontiguousarray(
            Wg.reshape(NF, 128, GD).transpose(1, 0, 2)
            .reshape(128, NF * GD).astype(BF))

    in_maps = []
    for c in range(NCORES):
        b, g = divmod(c, GH)
        xq_t, xk_t, xv_t = xP[b]
        in_maps.append({
            "xq": xq_t, "xk": xk_t, "xv": xv_t,
            "wq": w_chunks(Wq, g), "wk": w_chunks(Wk, g),
            "wv": w_chunks(Wv, g),
            "wo": np.ascontiguousarray(
                Wo[g * GD:(g + 1) * GD, :].astype(BF)).reshape(2, 128, D),
            "bqv": np.ascontiguousarray(
                bq[g * GD:(g + 1) * GD].reshape(2, 128).T),
        })

    LAST_RESULTS = run_bass_kernel_spmd(
        nc, in_maps, list(range(NCORES)),
        trace=bool(os.environ.get("KERNEL_TRACE")))
    res = LAST_RESULTS.results

    const_row = (bv @ Wo + bo).astype(np.float32)  # attn rows sum to 1
    full = np.empty((B, T, D), np.float32)
    for b in range(B):
        acc = res[b * GH]["out"].astype(np.float32)
        for g in range(1, GH):
            acc = acc + res[b * GH + g]["out"].astype(np.float32)
        full[b] = acc + const_row
    return full


# revision 40
# speedup vs baseline: 1.8209x; 1.0026x over previous
"""Trainium2 Bass kernel: 16-head MHA (B=2, T=2048, D=1024, d_k=64).

Sharding (8 NeuronCores): data-parallel over the batch (2) x tensor-parallel
over head groups (4 groups of 4 heads).  Core c handles batch b = c//4 and
heads [4g, 4g+4) with g = c%4.  Each core computes its partial output
    sum_{h in group} softmax((q Wq_h + bq_h)(k Wk_h)^T / 8) (v Wv_h) Wo_h
and the host sums the 4 partials per batch and adds the constant row
bo + bv @ Wo once.  bk is dropped: with the all-ones mask it shifts every
score row by a per-row constant, which softmax ignores exactly.

v2 design notes (vs the 391us baseline):
  * every matmul operand is bf16 (FWL weight loads, fp32 PSUM accumulate);
    output DMA'd as bf16 and upconverted host-side.
  * V is projected directly in [t, v-col] layout (stationary = x^T chunk,
    moving = Wv), killing the 32 PE transposes + drains of v1.
  * attention runs in 512-wide q stripes; per (stripe, head-pair, kt):
    2 scores MMs -> one [128,1024] fp32 PSUM tile, one ACT exp -> bf16,
    2 aV MMs accumulating into per-head [65,512] PSUM.  PSUM budget:
    scores dbuf 2x2 banks + O-accum 2 + filler 2 = 8 banks exactly, so
    scores(kt+1) / exp(kt) / aV(kt) pipeline without stalls.
  * Q projection of stripe s+1, the reciprocal dance, and the output
    projection of stripe s-1 are emitted as PE fillers inside the
    (ACT-bound) attention loop, keeping the PE dense so the HAM clock
    gate stays at 2.4 GHz instead of the baseline's 1.2 GHz cold clock.
  * softmax denominators: rowsums ride along as a 65th V_ext column; the
    reciprocal is computed partition-parallel by DVE 32-block transposing
    the per-head rowsum rows (heads pinned to partitions 0/32/64/96),
    one batched ACT Ln + Exp(-1), transposing back, and broadcasting
    across partitions with per-head rank-1 bf16 matmuls.
"""

import functools
import os
from collections import deque

import ml_dtypes
import numpy as np

import concourse.bass as bass
import concourse.mybir as mybir
import concourse.tile as tile
from concourse import bacc
from concourse.bass_utils import run_bass_kernel_spmd

F32 = mybir.dt.float32
F32R = mybir.dt.float32r
BF16 = mybir.dt.bfloat16
AFT = mybir.ActivationFunctionType
BF = ml_dtypes.bfloat16

D = 1024          # model dim
T = 2048          # sequence length
B = 2             # batch
HEADS = 16        # total heads
DK = 64           # head dim
NCORES = 8
GH = 4            # heads per core
GD = GH * DK      # 256 projection cols per core
NF = D // 128     # 8 contraction chunks
NKT = T // 128    # 16 k tiles
NQS = 4           # 512-wide q stripes
QW = T // NQS     # 512
SCALE = 1.0 / np.sqrt(np.float32(DK))  # 1/8

# Results of the last run (for test harness introspection: exec_time_ns etc.)
LAST_RESULTS = None


@functools.lru_cache(maxsize=1)
def _build_program():
    nc = bacc.Bacc("TRN2", target_bir_lowering=False, debug=False,
                   num_devices=NCORES)

    # host-packed activation layouts (see _pack_* in kernel()):
    #   xq[s]  = [128, NF*QW]  q-stripe s, chunk-major (8 KiB DMA lines)
    #   xk[qh] = [128, NF*QW]  k column-group qh, chunk-major
    #   xv[tb] = [128, NF*128] k-tile tb, chunk-major (2 KiB lines)
    xq = nc.declare_dram_parameter("xq", [NQS, 128, NF * QW], BF16,
                                   isOutput=False)
    xk = nc.declare_dram_parameter("xk", [4, 128, NF * QW], BF16,
                                   isOutput=False)
    xv = nc.declare_dram_parameter("xv", [NKT, 128, NF * 128], BF16,
                                   isOutput=False)
    wq = nc.declare_dram_parameter("wq", [128, NF * GD], BF16, isOutput=False)
    wk = nc.declare_dram_parameter("wk", [128, NF * GD], BF16, isOutput=False)
    wv = nc.declare_dram_parameter("wv", [128, NF * GD], BF16, isOutput=False)
    wo = nc.declare_dram_parameter("wo", [2, 128, D], BF16, isOutput=False)
    bqv = nc.declare_dram_parameter("bqv", [128, 2], F32, isOutput=False)
    out = nc.declare_dram_parameter("out", [T, D], BF16, isOutput=True)

    import contextlib
    with tile.TileContext(nc) as tc, contextlib.ExitStack() as _st:
        # ---- persistent pools -------------------------------------------
        def _pool(**kw):
            return _st.enter_context(tc.tile_pool(**kw))

        if True:
            kt_pool = _pool(name="kt", bufs=2)
            vext_pool = _pool(name="vext", bufs=NKT)
            qts_pool = _pool(name="qts", bufs=NQS * 2)
            ots_pool = _pool(name="ots", bufs=NQS * 2)
            w_pool = _pool(name="wts", bufs=3)
            wo_pool = _pool(name="wop", bufs=2)
            xq_pool = _pool(name="xq", bufs=NQS)
            xk_pool = _pool(name="xk", bufs=4)
            xv_pool = _pool(name="xv", bufs=NKT)
            const_pool = _pool(name="const", bufs=1)
            ones_f32 = const_pool.tile([128, GH], F32, tag="ones32")
            nc.gpsimd.memset(ones_f32[:], 1.0)
            ones_bf = const_pool.tile([128, DK], BF16, tag="onesbf")
            nc.gpsimd.memset(ones_bf[:], 1.0)
            # head-pair selectors: sel[hp][c, m] = (c == 32*(hp*2 + m//64)),
            # i.e. r_bc[m, :] = rinvT[32*head(m), :] after the C=128 matmul
            sel = [const_pool.tile([128, 128], BF16, tag=f"sel{hp}",
                                   name=f"sel{hp}") for hp in range(2)]
            for hp in range(2):
                nc.gpsimd.memset(sel[hp][:], 0.0)
                for hh in range(2):
                    c = 32 * (hp * 2 + hh)
                    nc.vector.tensor_copy(
                        sel[hp][c:c + 1, hh * DK:(hh + 1) * DK],
                        ones_bf[0:1, 0:DK])
            bqv_sb = const_pool.tile([128, 2], F32, tag="bqv")
            nc.sync.dma_start(bqv_sb[:], bqv[:])

            KT = [kt_pool.tile([128, T], BF16, tag="kt", name=f"kt{m}")
                  for m in range(2)]
            VE = [vext_pool.tile([128, GH * (DK + 1)], BF16, tag="vext",
                                 name=f"ve{i}") for i in range(NKT)]
            # per-stripe Q^T and O^T tiles (heads of pair hp stacked 64+64)
            QTs = [[qts_pool.tile([128, QW], BF16, tag="qts",
                                  name=f"qt{s}_{m}") for m in range(2)]
                   for s in range(NQS)]
            OTs = [[ots_pool.tile([128, QW], BF16, tag="ots",
                                  name=f"ot{s}_{m}") for m in range(2)]
                   for s in range(NQS)]
            WO = [wo_pool.tile([128, D], BF16, tag="wop", name=f"wo{m}")
                  for m in range(2)]

            wq_sb = w_pool.tile([128, NF * GD], BF16, tag="w", name="wq_sb")
            wk_sb = w_pool.tile([128, NF * GD], BF16, tag="w", name="wk_sb")
            wv_sb = w_pool.tile([128, NF * GD], BF16, tag="w", name="wv_sb")
            XQs = [xq_pool.tile([128, NF * QW], BF16, tag="xq",
                                name=f"xqs{s}") for s in range(NQS)]
            XKq = [xk_pool.tile([128, NF * QW], BF16, tag="xk",
                                name=f"xkq{i}") for i in range(4)]
            XVt = [xv_pool.tile([128, NF * 128], BF16, tag="xv",
                                name=f"xvt{i}") for i in range(NKT)]

            # V_ext ones columns (persistent; written once, no DMA dep)
            for tb in range(NKT):
                ve_r = VE[tb][:].rearrange("p (h x) -> p h x", x=DK + 1)
                nc.vector.tensor_copy(
                    ve_r[:, :, DK:DK + 1],
                    ones_f32[:].rearrange("p (h x) -> p h x", x=1))

            # ---- DMA in, ordered to match the projection filler schedule
            # (upfront: Q stripe 0, K group 0, V tiles 0-5; the rest lands
            # under stripe 0's attention).
            nc.sync.dma_start(wq_sb[:], wq[:])
            nc.sync.dma_start(XQs[0][:], xq[0])
            nc.sync.dma_start(wk_sb[:], wk[:])
            nc.sync.dma_start(XKq[0][:], xk[0])
            nc.sync.dma_start(wv_sb[:], wv[:])
            for tb in range(6):
                nc.sync.dma_start(XVt[tb][:], xv[tb])
            nc.sync.dma_start(XKq[1][:], xk[1])
            for tb in range(6, 10):
                nc.sync.dma_start(XVt[tb][:], xv[tb])
            nc.sync.dma_start(XKq[2][:], xk[2])
            nc.sync.dma_start(XKq[3][:], xk[3])
            for tb in range(10, NKT):
                nc.sync.dma_start(XVt[tb][:], xv[tb])
            for s in range(1, NQS):
                nc.sync.dma_start(XQs[s][:], xq[s])
            nc.sync.dma_start(WO[0][:], wo[0])
            nc.sync.dma_start(WO[1][:], wo[1])

            def q_project(s, m, ps_q, fc):
                nc.tensor.matmul(
                    ps_q[:],
                    wq_sb[:, fc * GD + m * 128:fc * GD + (m + 1) * 128],
                    XQs[s][:, fc * QW:(fc + 1) * QW],
                    start=(fc == 0), stop=(fc == NF - 1))
                if fc == NF - 1:
                    nc.vector.tensor_scalar_add(
                        QTs[s][m][:], ps_q[:], bqv_sb[:, m:m + 1])

            def k_project(qh, m, ps_k, fc):
                nc.tensor.matmul(
                    ps_k[:],
                    wk_sb[:, fc * GD + m * 128:fc * GD + (m + 1) * 128],
                    XKq[qh][:, fc * QW:(fc + 1) * QW],
                    start=(fc == 0), stop=(fc == NF - 1))
                if fc == NF - 1:
                    nc.vector.tensor_copy(
                        KT[m][:, qh * QW:(qh + 1) * QW], ps_k[:])

            def v_project(tb, ps_v, dc):
                nc.tensor.matmul(
                    ps_v[:, 0:GD],
                    XVt[tb][:, dc * 128:(dc + 1) * 128],
                    wv_sb[:, dc * GD:(dc + 1) * GD],
                    start=(dc == 0), stop=(dc == NF - 1))
                if dc == NF - 1:
                    ve_r = VE[tb][:].rearrange("p (h x) -> p h x", x=DK + 1)
                    nc.vector.tensor_copy(
                        ve_r[:, :, 0:DK],
                        ps_v[:, 0:GD].rearrange("p (h x) -> p h x", x=DK))

            # ---- phase A: warm the HAM clock gate with dummy matmuls
            # (no DMA dependency), then project Q stripe 0, K group 0 and
            # V tiles 0-5 as their inputs land.
            with tc.tile_pool(name="psA", bufs=8,
                              space=bass.MemorySpace.PSUM) as psA:
                # ~8.5us of dependency-free matmuls: warms the HAM clock
                # gate AND covers the ~7us DMA/preamble startup so real
                # projections start the moment their data lands.
                warm = psA.tile([128, QW], F32, tag="psA", name="warm")
                for i in range(150):
                    nc.tensor.matmul(
                        warm[0:DK, 0:DK], ones_bf[:, 0:DK],
                        ones_bf[:, 0:DK], start=True, stop=True)
                for m in range(2):
                    ps_q = psA.tile([128, QW], F32, tag="psA", name=f"psq{m}")
                    for fc in range(NF):
                        q_project(0, m, ps_q, fc)
                for m in range(2):
                    ps_k = psA.tile([128, QW], F32, tag="psA",
                                    name=f"psk0_{m}")
                    for fc in range(NF):
                        k_project(0, m, ps_k, fc)
                for tb in range(6):
                    ps_v = psA.tile([128, QW], F32, tag="psA",
                                    name=f"psv{tb}")
                    for dc in range(NF):
                        v_project(tb, ps_v, dc)

            # ---- phase B: striped attention with PE fillers -------------
            with contextlib.ExitStack() as _stB:
                def _poolB(**kw):
                    return _stB.enter_context(tc.tile_pool(**kw))

                es_pool = _poolB(name="ep", bufs=4)
                ub_pool = _poolB(name="ubp", bufs=8)
                rs_pool = _poolB(name="rsp", bufs=2)
                ob_pool = _poolB(name="obp", bufs=4)
                psS = _poolB(name="psS", bufs=2,
                             space=bass.MemorySpace.PSUM)
                psO = _poolB(name="psO", bufs=2,
                             space=bass.MemorySpace.PSUM)
                psF = _poolB(name="psF", bufs=2,
                             space=bass.MemorySpace.PSUM)
                ub_tiles = {}     # (qs, hp, hh) -> [64, 512] f32 tile
                rs_tiles = {}     # qs -> [128, 512] f32 rowsum-spread tile

                fstate = {}

                def qproj_fillers(s):
                    fs = []
                    for m in range(2):
                        def mk(mm, fc):
                            def f():
                                if fc == 0:
                                    fstate['q', mm] = psF.tile(
                                        [128, QW], F32, tag="psF",
                                        name=f"psq{s}_{mm}")
                                q_project(s, mm, fstate['q', mm], fc)
                            return f
                        for fc in range(NF):
                            fs.append(mk(m, fc))
                    return fs

                def kq_fillers(qh, m):
                    """K projection of column-group qh, head-pair tile m
                    (2 MMs per filler)."""
                    def mk(fp):
                        def f():
                            if fp == 0:
                                fstate['k', qh, m] = psF.tile(
                                    [128, QW], F32, tag="psF",
                                    name=f"psk{qh}_{m}")
                            for fc in (2 * fp, 2 * fp + 1):
                                k_project(qh, m, fstate['k', qh, m], fc)
                        return f
                    return [mk(fp) for fp in range(4)]

                def vtb_fillers(tb):
                    """V projection of k-tile tb (4 MMs per filler)."""
                    def mk(dp):
                        def f():
                            if dp == 0:
                                fstate['v', tb] = psF.tile(
                                    [128, QW], F32, tag="psF",
                                    name=f"psv{tb}")
                            for dc in range(4 * dp, 4 * dp + 4):
                                v_project(tb, fstate['v', tb], dc)
                        return f
                    return [mk(0), mk(1)]

                def recip_fillers(s, hps=(0, 1), state={}):
                    """Reciprocal dance + normalize for stripe s (rowsums
                    already collected at partitions 32h of rs_tiles[s]).
                    Split into [transpose, ln/exp/transpose, bcast...] so
                    the caller can space the ACT work away from its DVE
                    dependency in the filler stream."""
                    fs = []

                    def t1():
                        rsT = rs_pool.tile([128, QW], F32, tag="rsT",
                                           name=f"rsT{s}")
                        nc.vector.transpose(rsT[:], rs_tiles[s][:])
                        state[s] = rsT
                    fs.append(t1)

                    def t2():
                        rsT = state.pop(s)
                        nc.scalar.activation(rsT[:], rsT[:], AFT.Ln)
                        rinv = rs_pool.tile([128, QW], BF16, tag="rinv",
                                            name=f"rinv{s}")
                        nc.scalar.activation(rinv[:], rsT[:],
                                             AFT.Exp, scale=-1.0)
                        rinvT = rs_pool.tile([128, QW], BF16, tag="rinvT",
                                             name=f"rinvT{s}")
                        nc.vector.transpose(rinvT[:], rinv[:])
                        recip_fillers.rinvT = rinvT
                    fs.append(t2)

                    def mk_bcast(hp):
                        def f():
                            r_bc = psF.tile([128, QW], F32, tag="psF",
                                            name=f"rbc{s}_{hp}")
                            nc.tensor.matmul(
                                r_bc[:],
                                sel[hp][:],
                                recip_fillers.rinvT[:],
                                start=True, stop=True)
                            for hh in range(2):
                                nc.vector.tensor_mul(
                                    OTs[s][hp][hh * DK:(hh + 1) * DK, :],
                                    ub_tiles.pop((s, hp, hh))[0:DK, :],
                                    r_bc[hh * DK:(hh + 1) * DK, :])
                        return f
                    for hp in hps:
                        fs.append(mk_bcast(hp))
                    return fs
                recip_fillers.rinvT = None

                def outproj_fillers(s):
                    fs = []

                    def mk(tt, ei):
                        def f():
                            if ei == 0:
                                ob = ob_pool.tile([128, D], BF16, tag="ob",
                                                  name=f"ob{s}_{tt}")
                                outproj_fillers.ob = ob
                            ob = outproj_fillers.ob
                            f_ps = psF.tile([128, QW], F32, tag="psF",
                                            name=f"fps{s}_{tt}_{ei}")
                            for m in range(2):
                                nc.tensor.matmul(
                                    f_ps[:],
                                    OTs[s][m][:, tt * 128:(tt + 1) * 128],
                                    WO[m][:, ei * QW:(ei + 1) * QW],
                                    start=(m == 0), stop=(m == 1))
                            if s == NQS - 1:
                                # tail: ACT is idle, DVE is the tail chain
                                nc.scalar.activation(
                                    ob[:, ei * QW:(ei + 1) * QW], f_ps[:],
                                    AFT.Copy)
                            else:
                                nc.vector.tensor_copy(
                                    ob[:, ei * QW:(ei + 1) * QW], f_ps[:])
                            if ei == 1:
                                t0 = (s * 4 + tt) * 128
                                nc.sync.dma_start(out[t0:t0 + 128, :], ob[:])
                        return f
                    for tt in range(4):
                        for ei in range(2):
                            fs.append(mk(tt, ei))
                    return fs
                outproj_fillers.ob = None

                # flat (qs, hp, kt) stream: aV is emitted 1-3 steps behind
                # scores/exp so the FIFO PE queue never waits on an exp
                # before issuing independent scores work.  At block starts
                # the hold-back deepens to 3 so the previous block's DVE
                # drains (which gate aV(kt0) via o_ps buffer reuse) finish
                # under the run-ahead scores instead of stalling the PE.
                fillers = deque()
                pending = deque()  # (qs, hp, o_ps, es, kt)

                def flush_one():
                    pqs, php, po_ps, pes, pkt = pending.popleft()
                    for hh in range(2):
                        h = php * 2 + hh
                        nc.tensor.matmul(
                            po_ps[hh][0:DK + 1, :],
                            VE[pkt][:, h * (DK + 1):(h + 1) * (DK + 1)],
                            pes[:, hh * QW:(hh + 1) * QW],
                            start=(pkt == 0), stop=(pkt == NKT - 1))
                    if pkt == NKT - 1:
                        # drain O^T + rowsum row; heads at partitions 32h.
                        # The very last block's drains go on the otherwise
                        # idle ACT queue to shorten the serial tail.
                        last = pqs == NQS - 1 and php == 1
                        for hh in range(2):
                            h = php * 2 + hh
                            u = ub_pool.tile([128, QW], F32, tag="ub",
                                             name=f"ub{pqs}_{php}_{hh}")
                            if last and hh == 1:
                                nc.scalar.activation(
                                    u[0:DK, :], po_ps[hh][0:DK, :],
                                    AFT.Copy)
                            else:
                                nc.vector.tensor_copy(
                                    u[0:DK, :], po_ps[hh][0:DK, :])
                            ub_tiles[(pqs, php, hh)] = u
                            nc.vector.tensor_copy(
                                rs_tiles[pqs][32 * h:32 * h + 1, :],
                                po_ps[hh][DK:DK + 1, :])
                        if pqs == NQS - 1 and php == 0:
                            # last stripe: overlap hp0's half of the
                            # reciprocal dance under hp1's attention
                            rf = recip_fillers(pqs, hps=(0,))
                            fillers.append(rf[0])
                            fillers.extend([spacer] * 3)
                            fillers.extend(rf[1:])

                def spacer():
                    pass

                for qs in range(NQS):
                    rf = recip_fillers(qs - 1) if qs > 0 else []
                    qp = qproj_fillers(qs + 1) if qs < NQS - 1 else []
                    if qs == 0:
                        # remaining input projections ride along stripe 0
                        # (2 filler pops per kt), ordered so every tile's
                        # drain is emitted before its first consumer
                        fillers.extend(kq_fillers(1, 0))
                        for tb in range(6, 10):
                            fillers.extend(vtb_fillers(tb))
                        fillers.extend(kq_fillers(2, 0))
                        fillers.extend(kq_fillers(3, 0))
                        for tb in range(10, NKT):
                            fillers.extend(vtb_fillers(tb))
                        for qh in range(1, 4):
                            fillers.extend(kq_fillers(qh, 1))
                        fillers.extend(qp)
                    elif rf:
                        fillers.append(rf[0])       # DVE transpose
                        if qp:
                            fillers.extend(qp[0:8])  # qproj m0 (pins psF)
                        else:
                            fillers.extend([spacer] * 4)
                        fillers.extend(rf[1:])      # Ln/Exp + bcasts
                        fillers.extend(outproj_fillers(qs - 1))
                        fillers.extend(qp[8:16])    # qproj m1
                    else:
                        fillers.extend(qp)

                    rs_t = rs_pool.tile([128, QW], F32, tag="rs",
                                        name=f"rs{qs}")
                    nc.gpsimd.memset(rs_t[:], 1.0)
                    rs_tiles[qs] = rs_t

                    for hp in range(2):
                        o_ps = [psO.tile([128, QW], F32, tag="psO",
                                         name=f"o{qs}_{hp}_{i}")
                                for i in range(2)]
                        for kt in range(NKT):
                            sc = psS.tile([128, 2 * QW], F32, tag="psS",
                                          name=f"s{qs}_{hp}_{kt}")
                            for hh in range(2):
                                lo = hh * DK
                                nc.tensor.matmul(
                                    sc[:, hh * QW:(hh + 1) * QW],
                                    KT[hp][lo:lo + DK,
                                           kt * 128:(kt + 1) * 128],
                                    QTs[qs][hp][lo:lo + DK, :],
                                    start=True, stop=True)
                            es = es_pool.tile([128, 2 * QW], BF16, tag="es",
                                              name=f"e{qs}_{hp}_{kt}")
                            nc.scalar.activation(es[:], sc[:], AFT.Exp,
                                                 scale=float(SCALE))
                            # flush older blocks now; hold up to 3 of the
                            # current block while kt < 3
                            while pending and pending[0][0:2] != (qs, hp):
                                flush_one()
                            pending.append((qs, hp, o_ps, es, kt))
                            target = 3 if kt < 3 else 1
                            while len(pending) > target:
                                flush_one()
                            for _ in range(2 if qs == 0 else 1):
                                if fillers:
                                    fillers.popleft()()
                    # leftover fillers must land before the next stripe's
                    # scores read tiles they write (QTs of qs+1)
                    while fillers:
                        fillers.popleft()()

                # tail: flush last aV + drains, hp1 dance, outproj
                while pending:
                    flush_one()
                for f in recip_fillers(NQS - 1, hps=(1,)):
                    f()
                for f in outproj_fillers(NQS - 1):
                    f()

    from concourse.bacc import get_activation_tables
    import bass_rust as _br
    _combined = "natural_log_exp_and_others"
    _tabs = []
    for _name, _fns in get_activation_tables(nc.m.arch).items():
        if _name != _combined:
            _fns = _fns - {AFT.Exp, AFT.Ln}
        _tabs.append((_name, _fns))
    _br.insert_act_table_loads(nc, _tabs)
    nc.compile()
    return nc


def _numpy_reference(q, k, v, mask, Wq, bq, Wk, bk, Wv, bv, Wo, bo):
    """Fallback for a non-trivial mask (never hit with the stock inputs)."""
    Bn, Tn, _ = q.shape
    H, dk = HEADS, DK

    def split(x):
        return x.reshape(Bn, Tn, H, dk).transpose(0, 2, 1, 3)

    qh = split(q @ Wq + bq)
    kh = split(k @ Wk + bk)
    vh = split(v @ Wv + bv)
    s = np.einsum("bhqd,bhkd->bhqk", qh, kh) / np.sqrt(np.float32(dk))
    s = np.where(mask, s, -np.inf)
    s = s - s.max(axis=-1, keepdims=True)
    e = np.exp(s)
    a = e / e.sum(axis=-1, keepdims=True)
    o = np.einsum("bhqk,bhkd->bhqd", a, vh)
    o = o.transpose(0, 2, 1, 3).reshape(Bn, Tn, H * dk)
    return (o @ Wo + bo).astype(np.float32)


def kernel(q, k, v, mask, Wq, bq, Wk, bk, Wv, bv, Wo, bo):
    global LAST_RESULTS
    q = np.asarray(q, np.float32)
    k = np.asarray(k, np.float32)
    v = np.asarray(v, np.float32)
    mask = np.asarray(mask, bool)
    Wq, bq = np.asarray(Wq, np.float32), np.asarray(bq, np.float32)
    Wk, bk = np.asarray(Wk, np.float32), np.asarray(bk, np.float32)
    Wv, bv = np.asarray(Wv, np.float32), np.asarray(bv, np.float32)
    Wo, bo = np.asarray(Wo, np.float32), np.asarray(bo, np.float32)

    if not mask.all():
        return _numpy_reference(q, k, v, mask, Wq, bq, Wk, bk, Wv, bv, Wo, bo)

    nc = _build_program()

    # host-side sharding; activations packed chunk-major per column
    # group (see the dram parameter comments in _build_program)
    def pack_cols(xT_b, w):
        ng = T // w
        return np.ascontiguousarray(
            xT_b.reshape(NF, 128, ng, w).transpose(2, 1, 0, 3)
            .reshape(ng, 128, NF * w))

    xP = {}
    for b in range(B):
        xq_t, xk_t, xv_t = (x[b].T.astype(BF) for x in (q, k, v))
        xP[b] = (pack_cols(xq_t, QW), pack_cols(xk_t, QW),
                 pack_cols(xv_t, 128))

    def w_chunks(W, g):
        # (1024, 256) head-group slice -> [128, 8*256] chunk-major layout
        Wg = W[:, g * GD:(g + 1) * GD]
        return np.ascontiguousarray(
            Wg.reshape(NF, 128, GD).transpose(1, 0, 2)
            .reshape(128, NF * GD).astype(BF))

    in_maps = []
    for c in range(NCORES):
        b, g = divmod(c, GH)
        xq_t, xk_t, xv_t = xP[b]
        in_maps.append({
            "xq": xq_t, "xk": xk_t, "xv": xv_t,
            "wq": w_chunks(Wq, g), "wk": w_chunks(Wk, g),
            "wv": w_chunks(Wv, g),
            "wo": np.ascontiguousarray(
                Wo[g * GD:(g + 1) * GD, :].astype(BF)).reshape(2, 128, D),
            "bqv": np.ascontiguousarray(
                bq[g * GD:(g + 1) * GD].reshape(2, 128).T),
        })

    LAST_RESULTS = run_bass_kernel_spmd(
        nc, in_maps, list(range(NCORES)),
        trace=bool(os.environ.get("KERNEL_TRACE")))
    res = LAST_RESULTS.results

    const_row = (bv @ Wo + bo).astype(np.float32)  # attn rows sum to 1
    full = np.empty((B, T, D), np.float32)
    for b in range(B):
        acc = res[b * GH]["out"].astype(np.float32)
        for g in range(1, GH):
            acc = acc + res[b * GH + g]["out"].astype(np.float32)
        full[b] = acc + const_row
    return full
